# revision 4
# baseline (speedup 1.0000x reference)
"""MoE (Gemma-style 8-expert top-2) Trainium2 kernel — expert-quad / I-quarter,
mixed fp16/fp8 precision classes split by routing weight.

Strategy (8 NeuronCores):
  - Host: merge duplicate (token, expert) picks, build per-expert token lists.
    Split each expert's tokens into a hi class (route weight >= 0.4, fp16
    pipeline) and a lo class (fp8 e4m3 pipeline at 2x PE rate via DoubleRow
    matmuls).  Output error is dominated by the fp8 class but scales as
    theta^1.5 of the route-weight quantile -> ~1.5e-2 rel err at theta=0.4
    (measured on the reference data; threshold 2e-2).
  - Sort experts by count desc; group A = ranks {0,2,4,6}, group B =
    {1,3,5,7}.  Cores 0-3 serve group A (one I-quarter each), cores 4-7
    group B.  Slice widths W16[s]/W8[s] = ceil8(max over the two groups).
  - Device (per core): weights stream in fp16 once; fp8 copies are derived
    on-chip (x8 = fp8 scale 1; W8 = fp8(8*W16); h8 = fp8(8*h); psum_d8 =
    64*down, un-scaled on the host).  Phase 1 slice-serial; phase 2 emits
    fp16 partials, lo-class columns carry the 64x scale.
  - Host: combine with route weights (lo columns divided by 64).

Falls back to fp16-only programs when slice widths exceed limits.
"""

import numpy as np
import ml_dtypes

import concourse.bass as bass
import concourse.mybir as mybir
import concourse.tile as tile
from concourse import bacc


def _install_ntff_hook_shim():
    import sys
    import types

    try:
        import antenv

        try:
            from antenv import axon_hooks  # noqa: F401

            return
        except ImportError:
            pass
        mod = types.ModuleType("antenv.axon_hooks")
        mod._hook = None
        mod.set_axon_ntff_profile_hook = lambda h: setattr(mod, "_hook", h)
        mod.get_axon_ntff_profile_hook = lambda: mod._hook
        sys.modules["antenv.axon_hooks"] = mod
        antenv.axon_hooks = mod
        import os

        so_path = "/opt/axon/libaxon_pjrt.so"
        if os.path.exists(so_path):
            from trn_agent_boot.trn_boot import _ntff_profile_via_ctypes

            mod._hook = _ntff_profile_via_ctypes(so_path)
    except Exception:
        pass


_install_ntff_hook_shim()

from concourse.bass_utils import run_bass_kernel_spmd

H = 2048
I = 4096
E = 8
P = 128
KH = H // P  # 16 contraction chunks for gate/up
KHP = KH // 2  # 8 DoubleRow chunk-pairs
IQ = I // 4  # per-core I slice (1024)
MI4 = IQ // P  # 8 output tiles of I/4
KI4 = IQ // P  # 8 contraction chunks for down (per I-quarter)
KI4P = KI4 // 2  # 4 DoubleRow chunk-pairs for down
MH = H // P  # 16 output tiles of H
MI = I // P  # 32 output tiles of I (fallback single-expert program)
KI = I // P  # 32 contraction chunks for down (fallback)
NS = 4  # slices (experts) per core
F32 = mybir.dt.float32
F16 = mybir.dt.float16
F8 = mybir.dt.float8e4
FP8NP = ml_dtypes.float8_e4m3
DR = mybir.MatmulPerfMode.DoubleRow

THETA = 0.4  # route-weight cutoff: below -> fp8 class
SW = 8.0  # fp8 weight scale (dodges e4m3 subnormal floor)
YS8 = 64.0  # lo-class output carries SW*SW scale

LAST_RESULTS = None

_PROGRAM_CACHE: dict[tuple, "bass.Bass"] = {}


def _col_chunks(w):
    """Column chunks <=256 wide (DoubleRow moving free dim limit is 512)."""
    if w <= 256:
        return [(0, w)]
    half = -(-(w // 2) // 8) * 8
    return [(0, half), (half, w)]


def _build_program_quad_mix(W16, W8) -> "bass.Bass":
    """Bass program for one core: I/4 slice of a 4-expert MLP, 2 precision
    classes per slice.  Slice s has W16[s] fp16 columns and W8[s] fp8
    columns; y columns are [all fp16 slices | all fp8 slices]."""
    W16 = tuple(W16)
    W8 = tuple(W8)
    assert all(w % 8 == 0 and 8 <= w <= 512 for w in W16 + W8)
    o16 = [0]
    for w in W16:
        o16.append(o16[-1] + w)
    CT16 = o16[-1]
    o8 = [CT16]
    for w in W8:
        o8.append(o8[-1] + w)
    CT = o8[-1]

    nc = bacc.Bacc("TRN2", target_bir_lowering=False)

    XG = KH // 4
    xTs = [
        nc.dram_tensor(f"xT{s}", [XG, P, 4 * W16[s]], F16, kind="ExternalInput")
        for s in range(NS)
    ]
    xT8s = [
        nc.dram_tensor(f"xT8_{s}", [P, KHP * 2 * W8[s]], F8, kind="ExternalInput")
        for s in range(NS)
    ]
    Wgs = [
        nc.dram_tensor(f"Wg{s}", [MI4, P, KH * P], F16, kind="ExternalInput")
        for s in range(NS)
    ]
    Wus = [
        nc.dram_tensor(f"Wu{s}", [MI4, P, KH * P], F16, kind="ExternalInput")
        for s in range(NS)
    ]
    Wds = [
        nc.dram_tensor(f"Wd{s}", [MH, P, KI4 * P], F16, kind="ExternalInput")
        for s in range(NS)
    ]
    yT = nc.dram_tensor("yT", [H, CT], F16, kind="ExternalOutput")

    xT_r = [t.ap() for t in xTs]
    xT8_r = [t.ap() for t in xT8s]
    Wg_a = [t.ap() for t in Wgs]
    Wu_a = [t.ap() for t in Wus]
    Wd_a = [t.ap() for t in Wds]
    yT_r = yT.rearrange("(m p) c -> p m c", p=P)  # [128, 16, CT]

    gelu = mybir.ActivationFunctionType.Gelu_apprx_tanh
    copyf = mybir.ActivationFunctionType.Copy

    with tile.TileContext(nc) as tc:
        with (
            tc.tile_pool(name="xpool", bufs=1) as xpool,
            tc.tile_pool(name="hpool", bufs=1) as hpool,
            tc.tile_pool(name="wpool", bufs=3) as wpool,
            tc.tile_pool(name="w8pool", bufs=2) as w8pool,
            tc.tile_pool(name="wdpool", bufs=12) as wdpool,
            tc.tile_pool(name="wd8pool", bufs=6) as wd8pool,
            tc.tile_pool(name="tpool", bufs=3) as tpool,
            tc.tile_pool(name="warm", bufs=1) as warm_pool,
            tc.tile_pool(name="psum2", bufs=2, space="PSUM") as psum_pool,
        ):
            # --- PE warm-up: dummy matmuls cover BOTH the ~3.4us HAM clock
            # ramp AND the ~6us until the first weight/x bytes arrive.
            wz = warm_pool.tile([P, P], F16)
            xz = warm_pool.tile([P, W16[0]], F16)
            nc.vector.memset(wz, 0.0)
            nc.vector.memset(xz, 0.0)
            psum_w = psum_pool.tile([P, W16[0]], F32, tag="g")
            for _ in range(20):
                nc.tensor.matmul(psum_w, wz, xz, start=True, stop=True)

            xsb = [
                xpool.tile([P, KH, W16[s]], F16, tag=f"x{s}", name=f"xsb{s}")
                for s in range(NS)
            ]
            xsb8 = [
                xpool.tile([P, KHP, 2, W8[s]], F8, tag=f"x8{s}", name=f"xsb8{s}")
                for s in range(NS)
            ]
            hsb = [
                hpool.tile([P, KI4, W16[s]], F16, tag=f"h{s}", name=f"hsb{s}")
                for s in range(NS)
            ]
            hsb8 = [
                hpool.tile([P, KI4P, 2, W8[s]], F8, tag=f"h8{s}", name=f"hsb8{s}")
                for s in range(NS)
            ]

            def load_w(dram_ap, t, tag, pool=wpool, cols=KH * P, splits=1):
                wt = pool.tile([P, cols], F16, tag=tag, name=f"w_{tag}_{t}")
                step = cols // splits
                for sp in range(splits):
                    nc.sync.dma_start(
                        out=wt[:, sp * step : (sp + 1) * step],
                        in_=dram_ap[t, :, sp * step : (sp + 1) * step],
                    )
                return wt

            def load_x(s, g):
                nc.sync.dma_start(
                    out=xsb[s][:, 4 * g : 4 * (g + 1), :], in_=xT_r[s][g]
                )

            def load_x8(s):
                nc.sync.dma_start(
                    out=xsb8[s].rearrange("p a b c -> p (a b c)"), in_=xT8_r[s]
                )

            # Early DMA order: first pass's m0 weights + its x only.
            s0 = 0
            wg_t0 = load_w(Wg_a[s0], 0, "wg", splits=2)
            for g in range(XG):
                load_x(s0, g)
            load_x8(s0)
            wu_t0 = load_w(Wu_a[s0], 0, "wu")
            wg_t1 = load_w(Wg_a[s0], 1, "wg")
            wu_t1 = load_w(Wu_a[s0], 1, "wu")

            # ---- Phase 1: gateT/upT -> hT (both classes), slice-serial
            for si in range(NS):
                s = si
                for m in range(MI4):
                    if si == 0 and m == 0:
                        wg_t, wu_t = wg_t0, wu_t0
                    elif si == 0 and m == 1:
                        wg_t, wu_t = wg_t1, wu_t1
                    else:
                        wg_t = load_w(Wg_a[s], m, "wg")
                        wu_t = load_w(Wu_a[s], m, "wu")
                    if m == 2 and si < NS - 1:
                        for g in range(XG):
                            load_x(si + 1, g)
                        load_x8(si + 1)

                    # on-chip fp8 copies of this m-tile's weights (x SW)
                    wg8_t = w8pool.tile([P, KH * P], F8, tag="wg8")
                    wu8_t = w8pool.tile([P, KH * P], F8, tag="wu8")
                    nc.gpsimd.tensor_scalar_mul(wg8_t, wg_t, SW)
                    nc.vector.tensor_scalar_mul(wu8_t, wu_t, SW)

                    wg_v = wg_t.rearrange("p (k i) -> p k i", i=P)
                    wu_v = wu_t.rearrange("p (k i) -> p k i", i=P)
                    wg8_v = wg8_t.rearrange("p (a b i) -> p a b i", a=KHP, b=2)
                    wu8_v = wu8_t.rearrange("p (a b i) -> p a b i", a=KHP, b=2)

                    psum_g = psum_pool.tile([P, W16[s]], F32, tag="g")
                    psum_u = psum_pool.tile([P, W16[s]], F32, tag="u")
                    psum_g8 = psum_pool.tile([P, W8[s]], F32, tag="g8")
                    psum_u8 = psum_pool.tile([P, W8[s]], F32, tag="u8")
                    for k in range(KH):
                        nc.tensor.matmul(
                            psum_g, wg_v[:, k, :], xsb[s][:, k, :],
                            start=(k == 0), stop=(k == KH - 1),
                        )
                    for k in range(KH):
                        nc.tensor.matmul(
                            psum_u, wu_v[:, k, :], xsb[s][:, k, :],
                            start=(k == 0), stop=(k == KH - 1),
                        )
                    for c0, c1 in _col_chunks(W8[s]):
                        for kp in range(KHP):
                            nc.tensor.matmul(
                                psum_g8[:, c0:c1], wg8_v[:, kp, :, :],
                                xsb8[s][:, kp, :, c0:c1],
                                start=(kp == 0), stop=(kp == KHP - 1),
                                perf_mode=DR,
                            )
                    for c0, c1 in _col_chunks(W8[s]):
                        for kp in range(KHP):
                            nc.tensor.matmul(
                                psum_u8[:, c0:c1], wu8_v[:, kp, :, :],
                                xsb8[s][:, kp, :, c0:c1],
                                start=(kp == 0), stop=(kp == KHP - 1),
                                perf_mode=DR,
                            )
                    tg = tpool.tile([P, W16[s]], F32, tag="gelu")
                    nc.scalar.activation(tg, psum_g, gelu)
                    nc.vector.tensor_mul(hsb[s][:, m, :], tg, psum_u)
                    tg8 = tpool.tile([P, W8[s]], F32, tag="gelu8")
                    nc.scalar.activation(tg8, psum_g8, gelu, scale=1.0 / SW)
                    nc.vector.tensor_mul(
                        hsb8[s][:, m // 2, m % 2, :], tg8, psum_u8
                    )

            # ---- Phase 2: downT partials -> yT (fp16); lo columns = 64*down
            ptags = ["g", "u"]
            p8tags = ["g8", "u8"]
            pidx = 0
            for m2 in range(MH):
                wd_ts = [
                    load_w(Wd_a[s], m2, "wd", pool=wdpool, cols=KI4 * P)
                    for s in range(NS)
                ]
                wd8_ts = []
                for s in range(NS):
                    wd8_t = wd8pool.tile([P, KI4 * P], F8, tag="wd8")
                    if s % 2 == 0:
                        nc.gpsimd.tensor_scalar_mul(wd8_t, wd_ts[s], SW)
                    else:
                        nc.scalar.activation(wd8_t, wd_ts[s], copyf, scale=SW)
                    wd8_ts.append(wd8_t)

                ysb = tpool.tile([P, CT], F16, tag="ystage", bufs=2)
                for s in range(NS):
                    wd_v = wd_ts[s].rearrange("p (k i) -> p k i", i=P)
                    psum_d = psum_pool.tile([P, W16[s]], F32, tag=ptags[pidx % 2])
                    for k2 in range(KI4):
                        nc.tensor.matmul(
                            psum_d, wd_v[:, k2, :], hsb[s][:, k2, :],
                            start=(k2 == 0), stop=(k2 == KI4 - 1),
                        )
                    nc.vector.tensor_copy(
                        ysb[:, o16[s] : o16[s] + W16[s]], psum_d
                    )
                    pidx += 1
                for s in range(NS):
                    wd8_v = wd8_ts[s].rearrange(
                        "p (a b i) -> p a b i", a=KI4P, b=2
                    )
                    psum_d8 = psum_pool.tile(
                        [P, W8[s]], F32, tag=p8tags[pidx % 2]
                    )
                    for c0, c1 in _col_chunks(W8[s]):
                        for kp in range(KI4P):
                            nc.tensor.matmul(
                                psum_d8[:, c0:c1], wd8_v[:, kp, :, :],
                                hsb8[s][:, kp, :, c0:c1],
                                start=(kp == 0), stop=(kp == KI4P - 1),
                                perf_mode=DR,
                            )
                    nc.vector.tensor_copy(
                        ysb[:, o8[s] : o8[s] + W8[s]], psum_d8
                    )
                    pidx += 1
                    if m2 == MH - 1 and s == NS - 2:
                        # flush everything but the last fp8 slice early
                        nc.sync.dma_start(
                            out=yT_r[:, m2, 0 : o8[NS - 1]],
                            in_=ysb[:, 0 : o8[NS - 1]],
                        )
                if m2 < MH - 1:
                    nc.sync.dma_start(out=yT_r[:, m2, :], in_=ysb)
                else:
                    nc.sync.dma_start(
                        out=yT_r[:, m2, o8[NS - 1] : CT],
                        in_=ysb[:, o8[NS - 1] : CT],
                    )

    nc.compile()
    return nc


def _get_program_quad_mix(W16, W8) -> "bass.Bass":
    key = ("quadmix", tuple(W16), tuple(W8))
    if key not in _PROGRAM_CACHE:
        _PROGRAM_CACHE[key] = _build_program_quad_mix(W16, W8)
    return _PROGRAM_CACHE[key]


def _prep_w_gu_q(w):  # [H, IQ] f32 -> [MI4, P, KH*P] fp16
    return np.ascontiguousarray(
        w.astype(np.float16).reshape(KH, P, MI4, P).transpose(2, 1, 0, 3)
    ).reshape(MI4, P, KH * P)


def _prep_w_d_q(w):  # [IQ, H] f32 -> [MH, P, KI4*P] fp16
    return np.ascontiguousarray(
        w.astype(np.float16).reshape(KI4, P, MH, P).transpose(2, 1, 0, 3)
    ).reshape(MH, P, KI4 * P)


def _ceil8(n):
    return max(8, -(-n // 8) * 8)


def kernel(x, selected_experts, routing_weights, Wg, Wu, Wd):
    global LAST_RESULTS
    x = np.asarray(x, dtype=np.float32)
    se = np.asarray(selected_experts).astype(np.int64)
    rw = np.asarray(routing_weights).astype(np.float32)
    Wg = np.asarray(Wg, dtype=np.float32)
    Wu = np.asarray(Wu, dtype=np.float32)
    Wd = np.asarray(Wd, dtype=np.float32)

    T, K = se.shape
    assert x.shape == (T, H) and Wg.shape == (E, H, I) and Wd.shape == (E, I, H)

    # Dense route matrix, identical to the reference's scatter-add (merges
    # duplicate expert picks within a token by summing their weights).
    flat_t = np.repeat(np.arange(T), K)
    flat_e = se.ravel()
    route = np.zeros((T, E), np.float32)
    np.add.at(route, (flat_t, flat_e), rw.ravel())
    present = np.zeros((T, E), bool)
    present[flat_t, flat_e] = True

    idx_lists = [np.nonzero(present[:, e])[0] for e in range(E)]
    counts = np.array([len(ix) for ix in idx_lists])

    # Per-expert class split by route weight.
    ix16_l, ix8_l = [], []
    for e in range(E):
        ix = idx_lists[e]
        r = route[ix, e]
        lo = r < THETA
        ix16_l.append(ix[~lo])
        ix8_l.append(ix[lo])

    order = np.argsort(-counts, kind="stable")
    groups = [order[0::2], order[1::2]]  # ranks {0,2,4,6} and {1,3,5,7}
    W16 = [
        _ceil8(int(max(len(ix16_l[groups[0][s]]), len(ix16_l[groups[1][s]]))))
        for s in range(NS)
    ]
    W8 = [
        _ceil8(int(max(len(ix8_l[groups[0][s]]), len(ix8_l[groups[1][s]]))))
        for s in range(NS)
    ]
    if max(W16) <= 512 and max(W8) <= 512:
        return _kernel_quad_mix(
            x, route, ix16_l, ix8_l, groups, W16, W8, Wg, Wu, Wd
        )
    return _kernel_single(x, route, present, idx_lists, Wg, Wu, Wd)


def _kernel_quad_mix(x, route, ix16_l, ix8_l, groups, W16, W8, Wg, Wu, Wd):
    global LAST_RESULTS
    T = x.shape[0]
    o16 = np.concatenate([[0], np.cumsum(W16)])
    CT16 = int(o16[-1])
    o8 = np.concatenate([[CT16], CT16 + np.cumsum(W8)])

    nc = _get_program_quad_mix(W16, W8)
    xhalf = x.astype(np.float16)

    def pack_x16(ix, C):
        # [H, C] -> [XG=4, P, 4*C]: partition-major 4-chunk groups
        xt = np.zeros((H, C), np.float16)
        if len(ix):
            xt[:, : len(ix)] = xhalf[ix].T
        return np.ascontiguousarray(
            xt.reshape(4, 4, P, C).transpose(0, 2, 1, 3)
        ).reshape(4, P, 4 * C)

    def pack_x8(ix, C):
        # [H, C] fp8 -> [P, KHP*2*C]: (p, kp, i, c), k = kp*256 + i*128 + p
        xt = np.zeros((H, C), np.float32)
        if len(ix):
            xt[:, : len(ix)] = x[ix].T
        x8 = xt.astype(FP8NP)
        return np.ascontiguousarray(
            x8.reshape(KHP, 2, P, C).transpose(2, 0, 1, 3)
        ).reshape(P, KHP * 2 * C)

    in_maps = []
    for g in range(2):
        exps = [int(e) for e in groups[g]]
        xpacks = {}
        for s in range(NS):
            xpacks[f"xT{s}"] = pack_x16(ix16_l[exps[s]], W16[s])
            xpacks[f"xT8_{s}"] = pack_x8(ix8_l[exps[s]], W8[s])
        for q in range(4):
            sl = slice(q * IQ, (q + 1) * IQ)
            m = dict(xpacks)
            for s in range(NS):
                m[f"Wg{s}"] = _prep_w_gu_q(Wg[exps[s]][:, sl])
                m[f"Wu{s}"] = _prep_w_gu_q(Wu[exps[s]][:, sl])
                m[f"Wd{s}"] = _prep_w_d_q(Wd[exps[s]][sl, :])
            in_maps.append(m)
    res = run_bass_kernel_spmd(nc, in_maps, core_ids=list(range(E)))
    LAST_RESULTS = res

    out = np.zeros((T, H), np.float32)
    for g in range(2):
        exps = [int(e) for e in groups[g]]
        ysum = sum(
            res.results[4 * g + q]["yT"].astype(np.float32) for q in range(4)
        )  # [H, CT]
        for s in range(NS):
            ix = ix16_l[exps[s]]
            if len(ix):
                out[ix] += (
                    route[ix, exps[s]][:, None]
                    * ysum[:, o16[s] : o16[s] + len(ix)].T
                )
            ix = ix8_l[exps[s]]
            if len(ix):
                out[ix] += (
                    (route[ix, exps[s]] / YS8)[:, None]
                    * ysum[:, o8[s] : o8[s] + len(ix)].T
                )
    return out


def _build_program_single(C: int) -> "bass.Bass":
    """Fallback: one core = one expert's MLP on C tokens (fp16 only)."""
    assert C % 8 == 0 and 256 <= C <= 512

    nc = bacc.Bacc("TRN2", target_bir_lowering=False)

    XG = KH // 4
    xT = nc.dram_tensor("xT", [XG, P, 4 * C], F16, kind="ExternalInput")
    Wg = nc.dram_tensor("Wg", [MI, P, KH * P], F16, kind="ExternalInput")
    Wu = nc.dram_tensor("Wu", [MI, P, KH * P], F16, kind="ExternalInput")
    Wd = nc.dram_tensor("Wd", [MH, P, KI * P], F16, kind="ExternalInput")
    yT = nc.dram_tensor("yT", [H, C], F32, kind="ExternalOutput")

    xT_r = xT.ap()  # [XG, 128, 4*C]
    yT_r = yT.rearrange("(m p) c -> p m c", p=P)  # [128, 16, C]
    Wg_a, Wu_a, Wd_a = Wg.ap(), Wu.ap(), Wd.ap()

    gelu = mybir.ActivationFunctionType.Gelu_apprx_tanh

    with tile.TileContext(nc) as tc:
        with (
            tc.tile_pool(name="xpool", bufs=1) as xpool,
            tc.tile_pool(name="hpool", bufs=1) as hpool,
            tc.tile_pool(name="wpool", bufs=6) as wpool,
            tc.tile_pool(name="tpool", bufs=3) as tpool,
            tc.tile_pool(name="warm", bufs=1) as warm_pool,
            tc.tile_pool(name="psum2", bufs=2, space="PSUM") as psum_pool,
            tc.tile_pool(name="psumw", bufs=1, space="PSUM") as psum_warm,
        ):
            wz = warm_pool.tile([P, P], F16)
            xz = warm_pool.tile([P, C], F16)
            nc.vector.memset(wz, 0.0)
            nc.vector.memset(xz, 0.0)
            psum_w = psum_warm.tile([P, C], F32, tag="warm")
            for _ in range(13):
                nc.tensor.matmul(psum_w, wz, xz, start=True, stop=True)

            xsb = xpool.tile([P, KH, C], F16)
            hsb = hpool.tile([P, KI, C], F16)

            def load_w(dram_ap, t, tag, splits=1):
                wt = wpool.tile([P, KH * P], F16, tag=tag, name=f"w_{tag}_{t}")
                step = (KH * P) // splits
                for s in range(splits):
                    nc.sync.dma_start(
                        out=wt[:, s * step : (s + 1) * step],
                        in_=dram_ap[t, :, s * step : (s + 1) * step],
                    )
                return wt.rearrange("p (k i) -> p k i", i=P)

            def load_x(g):
                nc.sync.dma_start(out=xsb[:, 4 * g : 4 * (g + 1), :], in_=xT_r[g])

            wg_t0 = load_w(Wg_a, 0, "wg", splits=2)
            load_x(0)
            load_x(1)
            wu_t0 = load_w(Wu_a, 0, "wu", splits=2)
            load_x(2)
            load_x(3)
            wg_t1 = load_w(Wg_a, 1, "wg")
            wu_t1 = load_w(Wu_a, 1, "wu")

            for m in range(MI):
                if m == 0:
                    wg_t, wu_t = wg_t0, wu_t0
                elif m == 1:
                    wg_t, wu_t = wg_t1, wu_t1
                else:
                    wg_t = load_w(Wg_a, m, "wg")
                    wu_t = load_w(Wu_a, m, "wu")

                psum_g = psum_pool.tile([P, C], F32, tag="g")
                psum_u = psum_pool.tile([P, C], F32, tag="u")
                for k in range(KH):
                    nc.tensor.matmul(
                        psum_g, wg_t[:, k, :], xsb[:, k, :],
                        start=(k == 0), stop=(k == KH - 1),
                    )
                for k in range(KH):
                    nc.tensor.matmul(
                        psum_u, wu_t[:, k, :], xsb[:, k, :],
                        start=(k == 0), stop=(k == KH - 1),
                    )
                tg = tpool.tile([P, C], F32, tag="gelu")
                nc.scalar.activation(tg, psum_g, gelu)
                nc.vector.tensor_mul(hsb[:, m, :], tg, psum_u)

            for m2 in range(MH):
                wd_t = wpool.tile([P, KI * P], F16, tag="wd", name=f"w_wd_{m2}")
                nc.sync.dma_start(out=wd_t, in_=Wd_a[m2])
                wd_v = wd_t.rearrange("p (k i) -> p k i", i=P)
                if m2 < MH - 2:
                    pieces = [(0, C)]
                elif m2 == MH - 2:
                    pieces = [(0, C // 2), (C // 2, C)]
                else:
                    q = -(-C // 32) * 8
                    pieces = [(i * q, min((i + 1) * q, C)) for i in range(4)]
                    pieces = [(a, b) for a, b in pieces if b > a]
                ptags = ["d", "g"]
                for pi, (c0, c1) in enumerate(pieces):
                    psum_d = psum_pool.tile([P, c1 - c0], F32, tag=ptags[pi % 2])
                    for k2 in range(KI):
                        nc.tensor.matmul(
                            psum_d, wd_v[:, k2, :], hsb[:, k2, c0:c1],
                            start=(k2 == 0), stop=(k2 == KI - 1),
                        )
                    ysb = tpool.tile([P, c1 - c0], F32, tag="y")
                    nc.vector.tensor_copy(ysb, psum_d)
                    nc.sync.dma_start(out=yT_r[:, m2, c0:c1], in_=ysb)

    nc.compile()
    return nc


def _prep_w_gu(w):  # [H, I] f32 -> [MI, P, KH*P] fp16
    return np.ascontiguousarray(
        w.astype(np.float16).reshape(KH, P, MI, P).transpose(2, 1, 0, 3)
    ).reshape(MI, P, KH * P)


def _prep_w_d(w):  # [I, H] f32 -> [MH, P, KI*P] fp16
    return np.ascontiguousarray(
        w.astype(np.float16).reshape(KI, P, MH, P).transpose(2, 1, 0, 3)
    ).reshape(MH, P, KI * P)


def _get_program_single(C: int) -> "bass.Bass":
    key = ("single", C)
    if key not in _PROGRAM_CACHE:
        _PROGRAM_CACHE[key] = _build_program_single(C)
    return _PROGRAM_CACHE[key]


def _kernel_single(x, route, present, idx_lists, Wg, Wu, Wd):
    """Fallback: one expert per core, capacity-chunked (handles any skew)."""
    global LAST_RESULTS
    T = x.shape[0]
    chunked = [
        [ix[s : s + 512] for s in range(0, max(len(ix), 1), 512)]
        for ix in idx_lists
    ]
    n_pass = max(len(ch) for ch in chunked)

    out = np.zeros((T, H), np.float32)
    for p in range(n_pass):
        parts = [ch[p] if p < len(ch) else np.empty(0, np.int64) for ch in chunked]
        max_count = max(len(ix) for ix in parts)
        C = max(256, min(512, -(-max(max_count, 1) // 8) * 8))
        nc = _get_program_single(C)
        in_maps = []
        for e in range(E):
            ix = parts[e]
            xT_e = np.zeros((H, C), np.float16)
            if len(ix):
                xT_e[:, : len(ix)] = x[ix].T.astype(np.float16)
            xT_e = np.ascontiguousarray(
                xT_e.reshape(4, 4, P, C).transpose(0, 2, 1, 3)
            ).reshape(4, P, 4 * C)
            in_maps.append(
                {
                    "xT": xT_e,
                    "Wg": _prep_w_gu(Wg[e]),
                    "Wu": _prep_w_gu(Wu[e]),
                    "Wd": _prep_w_d(Wd[e]),
                }
            )
        res = run_bass_kernel_spmd(nc, in_maps, core_ids=list(range(E)))
        LAST_RESULTS = res
        for e in range(E):
            ix = parts[e]
            if len(ix) == 0:
                continue
            yT_e = res.results[e]["yT"]  # [H, C]
            out[ix] += route[ix, e][:, None] * yT_e[:, : len(ix)].T
    return out


# revision 6
# speedup vs baseline: 5.1294x; 5.1294x over previous
"""MoE (Gemma-style 8-expert top-2) Trainium2 kernel — expert-quad / I-quarter,
mixed fp16/fp8 precision classes split by routing weight.

Strategy (8 NeuronCores):
  - Host: merge duplicate (token, expert) picks, build per-expert token lists.
    Split each expert's tokens into a hi class (route weight >= 0.4, fp16
    pipeline) and a lo class (fp8 e4m3 pipeline at 2x PE rate via DoubleRow
    matmuls).  Output error is dominated by the fp8 class but scales as
    theta^1.5 of the route-weight quantile -> ~1.5e-2 rel err at theta=0.4
    (measured on the reference data; threshold 2e-2).
  - Sort experts by count desc; group A = ranks {0,2,4,6}, group B =
    {1,3,5,7}.  Cores 0-3 serve group A (one I-quarter each), cores 4-7
    group B.  Slice widths W16[s]/W8[s] = ceil8(max over the two groups).
  - Device (per core): weights stream in fp16 once; fp8 copies are derived
    on-chip (x8 = fp8 scale 1; W8 = fp8(8*W16); h8 = fp8(8*h); psum_d8 =
    64*down, un-scaled on the host).  Phase 1 slice-serial; phase 2 emits
    fp16 partials, lo-class columns carry the 64x scale.
  - Host: combine with route weights (lo columns divided by 64).

Falls back to fp16-only programs when slice widths exceed limits.
"""

import numpy as np
import ml_dtypes

import concourse.bass as bass
import concourse.mybir as mybir
import concourse.tile as tile
from concourse import bacc


def _install_ntff_hook_shim():
    import sys
    import types

    try:
        import antenv

        try:
            from antenv import axon_hooks  # noqa: F401

            return
        except ImportError:
            pass
        mod = types.ModuleType("antenv.axon_hooks")
        mod._hook = None
        mod.set_axon_ntff_profile_hook = lambda h: setattr(mod, "_hook", h)
        mod.get_axon_ntff_profile_hook = lambda: mod._hook
        sys.modules["antenv.axon_hooks"] = mod
        antenv.axon_hooks = mod
        import os

        so_path = "/opt/axon/libaxon_pjrt.so"
        if os.path.exists(so_path):
            from trn_agent_boot.trn_boot import _ntff_profile_via_ctypes

            mod._hook = _ntff_profile_via_ctypes(so_path)
    except Exception:
        pass


_install_ntff_hook_shim()

from concourse.bass_utils import run_bass_kernel_spmd

H = 2048
I = 4096
E = 8
P = 128
KH = H // P  # 16 contraction chunks for gate/up
KHP = KH // 2  # 8 DoubleRow chunk-pairs
IQ = I // 4  # per-core I slice (1024)
MI4 = IQ // P  # 8 output tiles of I/4
KI4 = IQ // P  # 8 contraction chunks for down (per I-quarter)
KI4P = KI4 // 2  # 4 DoubleRow chunk-pairs for down
MH = H // P  # 16 output tiles of H
MI = I // P  # 32 output tiles of I (fallback single-expert program)
KI = I // P  # 32 contraction chunks for down (fallback)
NS = 4  # slices (experts) per core
F32 = mybir.dt.float32
F16 = mybir.dt.float16
F8 = mybir.dt.float8e4
FP8NP = ml_dtypes.float8_e4m3
DR = mybir.MatmulPerfMode.DoubleRow

THETA = 0.4  # route-weight cutoff: below -> fp8 class
SW = 8.0  # fp8 weight scale (dodges e4m3 subnormal floor)
YS8 = 64.0  # lo-class output carries SW*SW scale

LAST_RESULTS = None

_PROGRAM_CACHE: dict[tuple, "bass.Bass"] = {}


def _col_chunks(w):
    """Column chunks <=256 wide (DoubleRow moving free dim limit is 512)."""
    if w <= 256:
        return [(0, w)]
    half = -(-(w // 2) // 8) * 8
    return [(0, half), (half, w)]


def _build_program_quad_mix(W16, W8) -> "bass.Bass":
    """Bass program for one core: I/4 slice of a 4-expert MLP, 2 precision
    classes per slice.  Slice s has W16[s] fp16 columns and W8[s] fp8
    columns; y columns are [all fp16 slices | all fp8 slices]."""
    W16 = tuple(W16)
    W8 = tuple(W8)
    assert all(w % 8 == 0 and 8 <= w <= 512 for w in W16 + W8)
    o16 = [0]
    for w in W16:
        o16.append(o16[-1] + w)
    CT16 = o16[-1]
    o8 = [CT16]
    for w in W8:
        o8.append(o8[-1] + w)
    CT = o8[-1]

    nc = bacc.Bacc("TRN2", target_bir_lowering=False)

    XG = KH // 4
    xTs = [
        nc.dram_tensor(f"xT{s}", [XG, P, 4 * W16[s]], F16, kind="ExternalInput")
        for s in range(NS)
    ]
    xT8s = [
        nc.dram_tensor(f"xT8_{s}", [P, KHP * 2 * W8[s]], F8, kind="ExternalInput")
        for s in range(NS)
    ]
    Wgs = [
        nc.dram_tensor(f"Wg{s}", [MI4, P, KH * P], F16, kind="ExternalInput")
        for s in range(NS)
    ]
    Wus = [
        nc.dram_tensor(f"Wu{s}", [MI4, P, KH * P], F16, kind="ExternalInput")
        for s in range(NS)
    ]
    Wds = [
        nc.dram_tensor(f"Wd{s}", [MH, P, KI4 * P], F16, kind="ExternalInput")
        for s in range(NS)
    ]
    yT = nc.dram_tensor("yT", [H, CT], F16, kind="ExternalOutput")

    xT_r = [t.ap() for t in xTs]
    xT8_r = [t.ap() for t in xT8s]
    Wg_a = [t.ap() for t in Wgs]
    Wu_a = [t.ap() for t in Wus]
    Wd_a = [t.ap() for t in Wds]
    yT_r = yT.rearrange("(m p) c -> p m c", p=P)  # [128, 16, CT]

    gelu = mybir.ActivationFunctionType.Gelu_apprx_tanh
    copyf = mybir.ActivationFunctionType.Copy

    with tile.TileContext(nc) as tc:
        with (
            tc.tile_pool(name="xpool", bufs=1) as xpool,
            tc.tile_pool(name="hpool", bufs=1) as hpool,
            tc.tile_pool(name="wpool", bufs=3) as wpool,
            tc.tile_pool(name="w8pool", bufs=2) as w8pool,
            tc.tile_pool(name="wdpool", bufs=12) as wdpool,
            tc.tile_pool(name="wd8pool", bufs=6) as wd8pool,
            tc.tile_pool(name="tpool", bufs=3) as tpool,
            tc.tile_pool(name="warm", bufs=1) as warm_pool,
            tc.tile_pool(name="psum2", bufs=2, space="PSUM") as psum_pool,
        ):
            # --- PE warm-up: dummy matmuls cover BOTH the ~3.4us HAM clock
            # ramp AND the ~6us until the first weight/x bytes arrive.
            wz = warm_pool.tile([P, P], F16)
            xz = warm_pool.tile([P, W16[0]], F16)
            nc.vector.memset(wz, 0.0)
            nc.vector.memset(xz, 0.0)
            psum_w = psum_pool.tile([P, W16[0]], F32, tag="g")
            for _ in range(20):
                nc.tensor.matmul(psum_w, wz, xz, start=True, stop=True)

            xsb = [
                xpool.tile([P, KH, W16[s]], F16, tag=f"x{s}", name=f"xsb{s}")
                for s in range(NS)
            ]
            xsb8 = [
                xpool.tile([P, KHP, 2, W8[s]], F8, tag=f"x8{s}", name=f"xsb8{s}")
                for s in range(NS)
            ]
            hsb = [
                hpool.tile([P, KI4, W16[s]], F16, tag=f"h{s}", name=f"hsb{s}")
                for s in range(NS)
            ]
            hsb8 = [
                hpool.tile([P, KI4P, 2, W8[s]], F8, tag=f"h8{s}", name=f"hsb8{s}")
                for s in range(NS)
            ]

            def load_w(dram_ap, t, tag, pool=wpool, cols=KH * P, splits=1):
                wt = pool.tile([P, cols], F16, tag=tag, name=f"w_{tag}_{t}")
                step = cols // splits
                for sp in range(splits):
                    nc.sync.dma_start(
                        out=wt[:, sp * step : (sp + 1) * step],
                        in_=dram_ap[t, :, sp * step : (sp + 1) * step],
                    )
                return wt

            def load_x(s, g):
                nc.sync.dma_start(
                    out=xsb[s][:, 4 * g : 4 * (g + 1), :], in_=xT_r[s][g]
                )

            def load_x8(s):
                nc.sync.dma_start(
                    out=xsb8[s].rearrange("p a b c -> p (a b c)"), in_=xT8_r[s]
                )

            # Early DMA order: first pass's m0 weights + its x only.
            s0 = 0
            wg_t0 = load_w(Wg_a[s0], 0, "wg", splits=2)
            for g in range(XG):
                load_x(s0, g)
            load_x8(s0)
            wu_t0 = load_w(Wu_a[s0], 0, "wu")
            wg_t1 = load_w(Wg_a[s0], 1, "wg")
            wu_t1 = load_w(Wu_a[s0], 1, "wu")

            # ---- Phase 1: gateT/upT -> hT (both classes), slice-serial
            for si in range(NS):
                s = si
                for m in range(MI4):
                    if si == 0 and m == 0:
                        wg_t, wu_t = wg_t0, wu_t0
                    elif si == 0 and m == 1:
                        wg_t, wu_t = wg_t1, wu_t1
                    else:
                        wg_t = load_w(Wg_a[s], m, "wg")
                        wu_t = load_w(Wu_a[s], m, "wu")
                    if m == 2 and si < NS - 1:
                        for g in range(XG):
                            load_x(si + 1, g)
                        load_x8(si + 1)

                    # on-chip fp8 copies of this m-tile's weights (x SW).
                    # Scalar engine only: DVE/GpSimd tensor ops collapse
                    # ~15x under PE+DMA SBUF load; scalar ACTIVATE stays at
                    # 1 elem/lane/cycle.
                    wg8_t = w8pool.tile([P, KH * P], F8, tag="wg8")
                    wu8_t = w8pool.tile([P, KH * P], F8, tag="wu8")
                    nc.scalar.activation(wg8_t, wg_t, copyf, scale=SW)
                    nc.scalar.activation(wu8_t, wu_t, copyf, scale=SW)

                    wg_v = wg_t.rearrange("p (k i) -> p k i", i=P)
                    wu_v = wu_t.rearrange("p (k i) -> p k i", i=P)
                    wg8_v = wg8_t.rearrange("p (a b i) -> p a b i", a=KHP, b=2)
                    wu8_v = wu8_t.rearrange("p (a b i) -> p a b i", a=KHP, b=2)

                    psum_g = psum_pool.tile([P, W16[s]], F32, tag="g")
                    psum_u = psum_pool.tile([P, W16[s]], F32, tag="u")
                    psum_g8 = psum_pool.tile([P, W8[s]], F32, tag="g8")
                    psum_u8 = psum_pool.tile([P, W8[s]], F32, tag="u8")
                    for k in range(KH):
                        nc.tensor.matmul(
                            psum_g, wg_v[:, k, :], xsb[s][:, k, :],
                            start=(k == 0), stop=(k == KH - 1),
                        )
                    for k in range(KH):
                        nc.tensor.matmul(
                            psum_u, wu_v[:, k, :], xsb[s][:, k, :],
                            start=(k == 0), stop=(k == KH - 1),
                        )
                    for c0, c1 in _col_chunks(W8[s]):
                        for kp in range(KHP):
                            nc.tensor.matmul(
                                psum_g8[:, c0:c1], wg8_v[:, kp, :, :],
                                xsb8[s][:, kp, :, c0:c1],
                                start=(kp == 0), stop=(kp == KHP - 1),
                                perf_mode=DR,
                            )
                    for c0, c1 in _col_chunks(W8[s]):
                        for kp in range(KHP):
                            nc.tensor.matmul(
                                psum_u8[:, c0:c1], wu8_v[:, kp, :, :],
                                xsb8[s][:, kp, :, c0:c1],
                                start=(kp == 0), stop=(kp == KHP - 1),
                                perf_mode=DR,
                            )
                    tg = tpool.tile([P, W16[s]], F32, tag="gelu")
                    nc.scalar.activation(tg, psum_g, gelu)
                    nc.vector.tensor_mul(hsb[s][:, m, :], tg, psum_u)
                    tg8 = tpool.tile([P, W8[s]], F32, tag="gelu8")
                    nc.scalar.activation(tg8, psum_g8, gelu, scale=1.0 / SW)
                    nc.vector.tensor_mul(
                        hsb8[s][:, m // 2, m % 2, :], tg8, psum_u8
                    )

            # ---- Phase 2: downT partials -> yT (fp16); lo columns = 64*down
            ptags = ["g", "u"]
            p8tags = ["g8", "u8"]
            pidx = 0
            for m2 in range(MH):
                wd_ts = [
                    load_w(Wd_a[s], m2, "wd", pool=wdpool, cols=KI4 * P)
                    for s in range(NS)
                ]
                wd8_ts = []
                for s in range(NS):
                    wd8_t = wd8pool.tile([P, KI4 * P], F8, tag="wd8")
                    nc.scalar.activation(wd8_t, wd_ts[s], copyf, scale=SW)
                    wd8_ts.append(wd8_t)

                ysb = tpool.tile([P, CT], F16, tag="ystage", bufs=2)
                for s in range(NS):
                    wd_v = wd_ts[s].rearrange("p (k i) -> p k i", i=P)
                    psum_d = psum_pool.tile([P, W16[s]], F32, tag=ptags[pidx % 2])
                    for k2 in range(KI4):
                        nc.tensor.matmul(
                            psum_d, wd_v[:, k2, :], hsb[s][:, k2, :],
                            start=(k2 == 0), stop=(k2 == KI4 - 1),
                        )
                    nc.vector.tensor_copy(
                        ysb[:, o16[s] : o16[s] + W16[s]], psum_d
                    )
                    pidx += 1
                for s in range(NS):
                    wd8_v = wd8_ts[s].rearrange(
                        "p (a b i) -> p a b i", a=KI4P, b=2
                    )
                    psum_d8 = psum_pool.tile(
                        [P, W8[s]], F32, tag=p8tags[pidx % 2]
                    )
                    for c0, c1 in _col_chunks(W8[s]):
                        for kp in range(KI4P):
                            nc.tensor.matmul(
                                psum_d8[:, c0:c1], wd8_v[:, kp, :, :],
                                hsb8[s][:, kp, :, c0:c1],
                                start=(kp == 0), stop=(kp == KI4P - 1),
                                perf_mode=DR,
                            )
                    nc.vector.tensor_copy(
                        ysb[:, o8[s] : o8[s] + W8[s]], psum_d8
                    )
                    pidx += 1
                    if m2 == MH - 1 and s == NS - 2:
                        # flush everything but the last fp8 slice early
                        nc.sync.dma_start(
                            out=yT_r[:, m2, 0 : o8[NS - 1]],
                            in_=ysb[:, 0 : o8[NS - 1]],
                        )
                if m2 < MH - 1:
                    nc.sync.dma_start(out=yT_r[:, m2, :], in_=ysb)
                else:
                    nc.sync.dma_start(
                        out=yT_r[:, m2, o8[NS - 1] : CT],
                        in_=ysb[:, o8[NS - 1] : CT],
                    )

    nc.compile()
    return nc


def _get_program_quad_mix(W16, W8) -> "bass.Bass":
    key = ("quadmix", tuple(W16), tuple(W8))
    if key not in _PROGRAM_CACHE:
        _PROGRAM_CACHE[key] = _build_program_quad_mix(W16, W8)
    return _PROGRAM_CACHE[key]


def _prep_w_gu_q(w):  # [H, IQ] f32 -> [MI4, P, KH*P] fp16
    return np.ascontiguousarray(
        w.astype(np.float16).reshape(KH, P, MI4, P).transpose(2, 1, 0, 3)
    ).reshape(MI4, P, KH * P)


def _prep_w_d_q(w):  # [IQ, H] f32 -> [MH, P, KI4*P] fp16
    return np.ascontiguousarray(
        w.astype(np.float16).reshape(KI4, P, MH, P).transpose(2, 1, 0, 3)
    ).reshape(MH, P, KI4 * P)


def _ceil8(n):
    return max(8, -(-n // 8) * 8)


def kernel(x, selected_experts, routing_weights, Wg, Wu, Wd):
    global LAST_RESULTS
    x = np.asarray(x, dtype=np.float32)
    se = np.asarray(selected_experts).astype(np.int64)
    rw = np.asarray(routing_weights).astype(np.float32)
    Wg = np.asarray(Wg, dtype=np.float32)
    Wu = np.asarray(Wu, dtype=np.float32)
    Wd = np.asarray(Wd, dtype=np.float32)

    T, K = se.shape
    assert x.shape == (T, H) and Wg.shape == (E, H, I) and Wd.shape == (E, I, H)

    # Dense route matrix, identical to the reference's scatter-add (merges
    # duplicate expert picks within a token by summing their weights).
    flat_t = np.repeat(np.arange(T), K)
    flat_e = se.ravel()
    route = np.zeros((T, E), np.float32)
    np.add.at(route, (flat_t, flat_e), rw.ravel())
    present = np.zeros((T, E), bool)
    present[flat_t, flat_e] = True

    idx_lists = [np.nonzero(present[:, e])[0] for e in range(E)]
    counts = np.array([len(ix) for ix in idx_lists])

    # Per-expert class split by route weight.
    ix16_l, ix8_l = [], []
    for e in range(E):
        ix = idx_lists[e]
        r = route[ix, e]
        lo = r < THETA
        ix16_l.append(ix[~lo])
        ix8_l.append(ix[lo])

    order = np.argsort(-counts, kind="stable")
    groups = [order[0::2], order[1::2]]  # ranks {0,2,4,6} and {1,3,5,7}
    W16 = [
        _ceil8(int(max(len(ix16_l[groups[0][s]]), len(ix16_l[groups[1][s]]))))
        for s in range(NS)
    ]
    W8 = [
        _ceil8(int(max(len(ix8_l[groups[0][s]]), len(ix8_l[groups[1][s]]))))
        for s in range(NS)
    ]
    if max(W16) <= 512 and max(W8) <= 512:
        return _kernel_quad_mix(
            x, route, ix16_l, ix8_l, groups, W16, W8, Wg, Wu, Wd
        )
    return _kernel_single(x, route, present, idx_lists, Wg, Wu, Wd)


def _kernel_quad_mix(x, route, ix16_l, ix8_l, groups, W16, W8, Wg, Wu, Wd):
    global LAST_RESULTS
    T = x.shape[0]
    o16 = np.concatenate([[0], np.cumsum(W16)])
    CT16 = int(o16[-1])
    o8 = np.concatenate([[CT16], CT16 + np.cumsum(W8)])

    nc = _get_program_quad_mix(W16, W8)
    xhalf = x.astype(np.float16)

    def pack_x16(ix, C):
        # [H, C] -> [XG=4, P, 4*C]: partition-major 4-chunk groups
        xt = np.zeros((H, C), np.float16)
        if len(ix):
            xt[:, : len(ix)] = xhalf[ix].T
        return np.ascontiguousarray(
            xt.reshape(4, 4, P, C).transpose(0, 2, 1, 3)
        ).reshape(4, P, 4 * C)

    def pack_x8(ix, C):
        # [H, C] fp8 -> [P, KHP*2*C]: (p, kp, i, c), k = kp*256 + i*128 + p
        xt = np.zeros((H, C), np.float32)
        if len(ix):
            xt[:, : len(ix)] = x[ix].T
        x8 = xt.astype(FP8NP)
        return np.ascontiguousarray(
            x8.reshape(KHP, 2, P, C).transpose(2, 0, 1, 3)
        ).reshape(P, KHP * 2 * C)

    in_maps = []
    for g in range(2):
        exps = [int(e) for e in groups[g]]
        xpacks = {}
        for s in range(NS):
            xpacks[f"xT{s}"] = pack_x16(ix16_l[exps[s]], W16[s])
            xpacks[f"xT8_{s}"] = pack_x8(ix8_l[exps[s]], W8[s])
        for q in range(4):
            sl = slice(q * IQ, (q + 1) * IQ)
            m = dict(xpacks)
            for s in range(NS):
                m[f"Wg{s}"] = _prep_w_gu_q(Wg[exps[s]][:, sl])
                m[f"Wu{s}"] = _prep_w_gu_q(Wu[exps[s]][:, sl])
                m[f"Wd{s}"] = _prep_w_d_q(Wd[exps[s]][sl, :])
            in_maps.append(m)
    res = run_bass_kernel_spmd(nc, in_maps, core_ids=list(range(E)))
    LAST_RESULTS = res

    out = np.zeros((T, H), np.float32)
    for g in range(2):
        exps = [int(e) for e in groups[g]]
        ysum = sum(
            res.results[4 * g + q]["yT"].astype(np.float32) for q in range(4)
        )  # [H, CT]
        for s in range(NS):
            ix = ix16_l[exps[s]]
            if len(ix):
                out[ix] += (
                    route[ix, exps[s]][:, None]
                    * ysum[:, o16[s] : o16[s] + len(ix)].T
                )
            ix = ix8_l[exps[s]]
            if len(ix):
                out[ix] += (
                    (route[ix, exps[s]] / YS8)[:, None]
                    * ysum[:, o8[s] : o8[s] + len(ix)].T
                )
    return out


def _build_program_single(C: int) -> "bass.Bass":
    """Fallback: one core = one expert's MLP on C tokens (fp16 only)."""
    assert C % 8 == 0 and 256 <= C <= 512

    nc = bacc.Bacc("TRN2", target_bir_lowering=False)

    XG = KH // 4
    xT = nc.dram_tensor("xT", [XG, P, 4 * C], F16, kind="ExternalInput")
    Wg = nc.dram_tensor("Wg", [MI, P, KH * P], F16, kind="ExternalInput")
    Wu = nc.dram_tensor("Wu", [MI, P, KH * P], F16, kind="ExternalInput")
    Wd = nc.dram_tensor("Wd", [MH, P, KI * P], F16, kind="ExternalInput")
    yT = nc.dram_tensor("yT", [H, C], F32, kind="ExternalOutput")

    xT_r = xT.ap()  # [XG, 128, 4*C]
    yT_r = yT.rearrange("(m p) c -> p m c", p=P)  # [128, 16, C]
    Wg_a, Wu_a, Wd_a = Wg.ap(), Wu.ap(), Wd.ap()

    gelu = mybir.ActivationFunctionType.Gelu_apprx_tanh

    with tile.TileContext(nc) as tc:
        with (
            tc.tile_pool(name="xpool", bufs=1) as xpool,
            tc.tile_pool(name="hpool", bufs=1) as hpool,
            tc.tile_pool(name="wpool", bufs=6) as wpool,
            tc.tile_pool(name="tpool", bufs=3) as tpool,
            tc.tile_pool(name="warm", bufs=1) as warm_pool,
            tc.tile_pool(name="psum2", bufs=2, space="PSUM") as psum_pool,
            tc.tile_pool(name="psumw", bufs=1, space="PSUM") as psum_warm,
        ):
            wz = warm_pool.tile([P, P], F16)
            xz = warm_pool.tile([P, C], F16)
            nc.vector.memset(wz, 0.0)
            nc.vector.memset(xz, 0.0)
            psum_w = psum_warm.tile([P, C], F32, tag="warm")
            for _ in range(13):
                nc.tensor.matmul(psum_w, wz, xz, start=True, stop=True)

            xsb = xpool.tile([P, KH, C], F16)
            hsb = hpool.tile([P, KI, C], F16)

            def load_w(dram_ap, t, tag, splits=1):
                wt = wpool.tile([P, KH * P], F16, tag=tag, name=f"w_{tag}_{t}")
                step = (KH * P) // splits
                for s in range(splits):
                    nc.sync.dma_start(
                        out=wt[:, s * step : (s + 1) * step],
                        in_=dram_ap[t, :, s * step : (s + 1) * step],
                    )
                return wt.rearrange("p (k i) -> p k i", i=P)

            def load_x(g):
                nc.sync.dma_start(out=xsb[:, 4 * g : 4 * (g + 1), :], in_=xT_r[g])

            wg_t0 = load_w(Wg_a, 0, "wg", splits=2)
            load_x(0)
            load_x(1)
            wu_t0 = load_w(Wu_a, 0, "wu", splits=2)
            load_x(2)
            load_x(3)
            wg_t1 = load_w(Wg_a, 1, "wg")
            wu_t1 = load_w(Wu_a, 1, "wu")

            for m in range(MI):
                if m == 0:
                    wg_t, wu_t = wg_t0, wu_t0
                elif m == 1:
                    wg_t, wu_t = wg_t1, wu_t1
                else:
                    wg_t = load_w(Wg_a, m, "wg")
                    wu_t = load_w(Wu_a, m, "wu")

                psum_g = psum_pool.tile([P, C], F32, tag="g")
                psum_u = psum_pool.tile([P, C], F32, tag="u")
                for k in range(KH):
                    nc.tensor.matmul(
                        psum_g, wg_t[:, k, :], xsb[:, k, :],
                        start=(k == 0), stop=(k == KH - 1),
                    )
                for k in range(KH):
                    nc.tensor.matmul(
                        psum_u, wu_t[:, k, :], xsb[:, k, :],
                        start=(k == 0), stop=(k == KH - 1),
                    )
                tg = tpool.tile([P, C], F32, tag="gelu")
                nc.scalar.activation(tg, psum_g, gelu)
                nc.vector.tensor_mul(hsb[:, m, :], tg, psum_u)

            for m2 in range(MH):
                wd_t = wpool.tile([P, KI * P], F16, tag="wd", name=f"w_wd_{m2}")
                nc.sync.dma_start(out=wd_t, in_=Wd_a[m2])
                wd_v = wd_t.rearrange("p (k i) -> p k i", i=P)
                if m2 < MH - 2:
                    pieces = [(0, C)]
                elif m2 == MH - 2:
                    pieces = [(0, C // 2), (C // 2, C)]
                else:
                    q = -(-C // 32) * 8
                    pieces = [(i * q, min((i + 1) * q, C)) for i in range(4)]
                    pieces = [(a, b) for a, b in pieces if b > a]
                ptags = ["d", "g"]
                for pi, (c0, c1) in enumerate(pieces):
                    psum_d = psum_pool.tile([P, c1 - c0], F32, tag=ptags[pi % 2])
                    for k2 in range(KI):
                        nc.tensor.matmul(
                            psum_d, wd_v[:, k2, :], hsb[:, k2, c0:c1],
                            start=(k2 == 0), stop=(k2 == KI - 1),
                        )
                    ysb = tpool.tile([P, c1 - c0], F32, tag="y")
                    nc.vector.tensor_copy(ysb, psum_d)
                    nc.sync.dma_start(out=yT_r[:, m2, c0:c1], in_=ysb)

    nc.compile()
    return nc


def _prep_w_gu(w):  # [H, I] f32 -> [MI, P, KH*P] fp16
    return np.ascontiguousarray(
        w.astype(np.float16).reshape(KH, P, MI, P).transpose(2, 1, 0, 3)
    ).reshape(MI, P, KH * P)


def _prep_w_d(w):  # [I, H] f32 -> [MH, P, KI*P] fp16
    return np.ascontiguousarray(
        w.astype(np.float16).reshape(KI, P, MH, P).transpose(2, 1, 0, 3)
    ).reshape(MH, P, KI * P)


def _get_program_single(C: int) -> "bass.Bass":
    key = ("single", C)
    if key not in _PROGRAM_CACHE:
        _PROGRAM_CACHE[key] = _build_program_single(C)
    return _PROGRAM_CACHE[key]


def _kernel_single(x, route, present, idx_lists, Wg, Wu, Wd):
    """Fallback: one expert per core, capacity-chunked (handles any skew)."""
    global LAST_RESULTS
    T = x.shape[0]
    chunked = [
        [ix[s : s + 512] for s in range(0, max(len(ix), 1), 512)]
        for ix in idx_lists
    ]
    n_pass = max(len(ch) for ch in chunked)

    out = np.zeros((T, H), np.float32)
    for p in range(n_pass):
        parts = [ch[p] if p < len(ch) else np.empty(0, np.int64) for ch in chunked]
        max_count = max(len(ix) for ix in parts)
        C = max(256, min(512, -(-max(max_count, 1) // 8) * 8))
        nc = _get_program_single(C)
        in_maps = []
        for e in range(E):
            ix = parts[e]
            xT_e = np.zeros((H, C), np.float16)
            if len(ix):
                xT_e[:, : len(ix)] = x[ix].T.astype(np.float16)
            xT_e = np.ascontiguousarray(
                xT_e.reshape(4, 4, P, C).transpose(0, 2, 1, 3)
            ).reshape(4, P, 4 * C)
            in_maps.append(
                {
                    "xT": xT_e,
                    "Wg": _prep_w_gu(Wg[e]),
                    "Wu": _prep_w_gu(Wu[e]),
                    "Wd": _prep_w_d(Wd[e]),
                }
            )
        res = run_bass_kernel_spmd(nc, in_maps, core_ids=list(range(E)))
        LAST_RESULTS = res
        for e in range(E):
            ix = parts[e]
            if len(ix) == 0:
                continue
            yT_e = res.results[e]["yT"]  # [H, C]
            out[ix] += route[ix, e][:, None] * yT_e[:, : len(ix)].T
    return out


# revision 13
# speedup vs baseline: 5.2822x; 1.0298x over previous
"""MoE (Gemma-style 8-expert top-2) Trainium2 kernel — expert-quad / I-quarter,
mixed fp16/fp8 precision classes split by routing weight.

Strategy (8 NeuronCores):
  - Host: merge duplicate (token, expert) picks, build per-expert token lists.
    Split each expert's tokens into a hi class (route weight >= 0.4, fp16
    pipeline) and a lo class (fp8 e4m3 pipeline at 2x PE rate via DoubleRow
    matmuls).  Output error is dominated by the fp8 class but scales as
    theta^1.5 of the route-weight quantile -> ~1.5e-2 rel err at theta=0.4
    (measured on the reference data; threshold 2e-2).
  - Sort experts by count desc; group A = ranks {0,2,4,6}, group B =
    {1,3,5,7}.  Cores 0-3 serve group A (one I-quarter each), cores 4-7
    group B.  Slice widths W16[s]/W8[s] = ceil8(max over the two groups).
  - Device (per core): weights stream in fp16 once; fp8 copies are derived
    on-chip (x8 = fp8 scale 1; W8 = fp8(8*W16); h8 = fp8(8*h); psum_d8 =
    64*down, un-scaled on the host).  Phase 1 slice-serial; phase 2 emits
    fp16 partials, lo-class columns carry the 64x scale.
  - Host: combine with route weights (lo columns divided by 64).

Falls back to fp16-only programs when slice widths exceed limits.
"""

import numpy as np
import ml_dtypes

import concourse.bass as bass
import concourse.mybir as mybir
import concourse.tile as tile
from concourse import bacc


def _install_ntff_hook_shim():
    import sys
    import types

    try:
        import antenv

        try:
            from antenv import axon_hooks  # noqa: F401

            return
        except ImportError:
            pass
        mod = types.ModuleType("antenv.axon_hooks")
        mod._hook = None
        mod.set_axon_ntff_profile_hook = lambda h: setattr(mod, "_hook", h)
        mod.get_axon_ntff_profile_hook = lambda: mod._hook
        sys.modules["antenv.axon_hooks"] = mod
        antenv.axon_hooks = mod
        import os

        so_path = "/opt/axon/libaxon_pjrt.so"
        if os.path.exists(so_path):
            from trn_agent_boot.trn_boot import _ntff_profile_via_ctypes

            mod._hook = _ntff_profile_via_ctypes(so_path)
    except Exception:
        pass


_install_ntff_hook_shim()

from concourse.bass_utils import run_bass_kernel_spmd

H = 2048
I = 4096
E = 8
P = 128
KH = H // P  # 16 contraction chunks for gate/up
KHP = KH // 2  # 8 DoubleRow chunk-pairs
IQ = I // 4  # per-core I slice (1024)
MI4 = IQ // P  # 8 output tiles of I/4
KI4 = IQ // P  # 8 contraction chunks for down (per I-quarter)
KI4P = KI4 // 2  # 4 DoubleRow chunk-pairs for down
MH = H // P  # 16 output tiles of H
MI = I // P  # 32 output tiles of I (fallback single-expert program)
KI = I // P  # 32 contraction chunks for down (fallback)
NS = 4  # slices (experts) per core
F32 = mybir.dt.float32
F16 = mybir.dt.float16
F8 = mybir.dt.float8e4
FP8NP = ml_dtypes.float8_e4m3
DR = mybir.MatmulPerfMode.DoubleRow

THETA = 0.45  # route-weight cutoff: below -> fp8 class
SW = 8.0  # fp8 weight scale (dodges e4m3 subnormal floor)
YS8 = 64.0  # lo-class output carries SW*SW scale

LAST_RESULTS = None

_PROGRAM_CACHE: dict[tuple, "bass.Bass"] = {}


def _col_chunks(w):
    """Column chunks <=256 wide (DoubleRow moving free dim limit is 512)."""
    if w <= 256:
        return [(0, w)]
    half = -(-(w // 2) // 8) * 8
    return [(0, half), (half, w)]


def _build_program_quad_mix(W16, W8) -> "bass.Bass":
    """Bass program for one core: I/4 slice of a 4-expert MLP, 2 precision
    classes per slice.  Slice s has W16[s] fp16 columns and W8[s] fp8
    columns; y columns are [all fp16 slices | all fp8 slices]."""
    W16 = tuple(W16)
    W8 = tuple(W8)
    assert all(w % 8 == 0 and 8 <= w <= 512 for w in W16 + W8)
    o16 = [0]
    for w in W16:
        o16.append(o16[-1] + w)
    CT16 = o16[-1]
    o8 = [CT16]
    for w in W8:
        o8.append(o8[-1] + w)
    CT = o8[-1]

    nc = bacc.Bacc("TRN2", target_bir_lowering=False)

    XG = KH // 4
    # Slice 0's x streams in the early window: 4-chunk layout so the first
    # k-chunks' semaphore fires before the whole slab lands.  Later slices
    # prefetch during compute: flat partition-major, one trigger each.
    xTs = [
        nc.dram_tensor(
            f"xT{s}",
            [XG, P, 4 * W16[s]] if s == 0 else [P, KH * W16[s]],
            F16,
            kind="ExternalInput",
        )
        for s in range(NS)
    ]
    xT8s = [
        nc.dram_tensor(f"xT8_{s}", [P, KHP * 2 * W8[s]], F8, kind="ExternalInput")
        for s in range(NS)
    ]
    Wgs = [
        nc.dram_tensor(f"Wg{s}", [MI4, P, KH * P], F16, kind="ExternalInput")
        for s in range(NS)
    ]
    Wus = [
        nc.dram_tensor(f"Wu{s}", [MI4, P, KH * P], F16, kind="ExternalInput")
        for s in range(NS)
    ]
    Wds = [
        nc.dram_tensor(f"Wd{s}", [MH, P, KI4 * P], F16, kind="ExternalInput")
        for s in range(NS)
    ]
    yT = nc.dram_tensor("yT", [H, CT], F16, kind="ExternalOutput")

    xT_r = [t.ap() for t in xTs]
    xT8_r = [t.ap() for t in xT8s]
    Wg_a = [t.ap() for t in Wgs]
    Wu_a = [t.ap() for t in Wus]
    Wd_a = [t.ap() for t in Wds]
    yT_r = yT.rearrange("(m p) c -> p m c", p=P)  # [128, 16, CT]

    gelu = mybir.ActivationFunctionType.Gelu_apprx_tanh
    copyf = mybir.ActivationFunctionType.Copy

    with tile.TileContext(nc) as tc:
        with (
            tc.tile_pool(name="xpool", bufs=1) as xpool,
            tc.tile_pool(name="hpool", bufs=1) as hpool,
            tc.tile_pool(name="wpool", bufs=3) as wpool,
            tc.tile_pool(name="w8pool", bufs=2) as w8pool,
            tc.tile_pool(name="wdpool", bufs=12) as wdpool,
            tc.tile_pool(name="wd8pool", bufs=6) as wd8pool,
            tc.tile_pool(name="tpool", bufs=3) as tpool,
            tc.tile_pool(name="warm", bufs=1) as warm_pool,
            tc.tile_pool(name="psum2", bufs=2, space="PSUM") as psum_pool,
        ):
            # --- PE warm-up: dummy matmuls cover BOTH the ~3.4us HAM clock
            # ramp AND the ~6us until the first weight/x bytes arrive.
            wz = warm_pool.tile([P, P], F16)
            xz = warm_pool.tile([P, W16[0]], F16)
            nc.vector.memset(wz, 0.0)
            nc.vector.memset(xz, 0.0)
            psum_w = psum_pool.tile([P, W16[0]], F32, tag="g")
            for _ in range(20):
                nc.tensor.matmul(psum_w, wz, xz, start=True, stop=True)

            xsb = [
                xpool.tile([P, KH, W16[s]], F16, tag=f"x{s}", name=f"xsb{s}")
                for s in range(NS)
            ]
            xsb8 = [
                xpool.tile([P, KHP, 2, W8[s]], F8, tag=f"x8{s}", name=f"xsb8{s}")
                for s in range(NS)
            ]
            hsb = [
                hpool.tile([P, KI4, W16[s]], F16, tag=f"h{s}", name=f"hsb{s}")
                for s in range(NS)
            ]
            hsb8 = [
                hpool.tile([P, KI4P, 2, W8[s]], F8, tag=f"h8{s}", name=f"hsb8{s}")
                for s in range(NS)
            ]

            def load_w(dram_ap, t, tag, pool=wpool, cols=KH * P, splits=1):
                wt = pool.tile([P, cols], F16, tag=tag, name=f"w_{tag}_{t}")
                step = cols // splits
                for sp in range(splits):
                    nc.sync.dma_start(
                        out=wt[:, sp * step : (sp + 1) * step],
                        in_=dram_ap[t, :, sp * step : (sp + 1) * step],
                    )
                return wt

            def load_x(s):
                if s == 0:
                    for g in range(XG):
                        nc.sync.dma_start(
                            out=xsb[s][:, 4 * g : 4 * (g + 1), :], in_=xT_r[s][g]
                        )
                else:
                    nc.sync.dma_start(
                        out=xsb[s].rearrange("p a b -> p (a b)"), in_=xT_r[s]
                    )

            def load_x8(s):
                nc.sync.dma_start(
                    out=xsb8[s].rearrange("p a b c -> p (a b c)"), in_=xT8_r[s]
                )

            # Early DMA order: first pass's m0 weights + its x only.
            s0 = 0
            wg_t0 = load_w(Wg_a[s0], 0, "wg", splits=2)
            load_x(s0)
            load_x8(s0)
            wu_t0 = load_w(Wu_a[s0], 0, "wu")
            wg_t1 = load_w(Wg_a[s0], 1, "wg")
            wu_t1 = load_w(Wu_a[s0], 1, "wu")

            # ---- Phase 1: gateT/upT -> hT (both classes), slice-serial
            for si in range(NS):
                s = si
                for m in range(MI4):
                    if si == 0 and m == 0:
                        wg_t, wu_t = wg_t0, wu_t0
                    elif si == 0 and m == 1:
                        wg_t, wu_t = wg_t1, wu_t1
                    else:
                        wg_t = load_w(Wg_a[s], m, "wg")
                        wu_t = load_w(Wu_a[s], m, "wu")
                    if m == 2 and si < NS - 1:
                        load_x(si + 1)
                        load_x8(si + 1)

                    # on-chip fp8 copies of this m-tile's weights (x SW).
                    # Scalar engine only: DVE/GpSimd tensor ops collapse
                    # ~15x under PE+DMA SBUF load; scalar ACTIVATE stays at
                    # 1 elem/lane/cycle.
                    wg8_t = w8pool.tile([P, KH * P], F8, tag="wg8")
                    wu8_t = w8pool.tile([P, KH * P], F8, tag="wu8")
                    nc.scalar.activation(wg8_t, wg_t, copyf, scale=SW)
                    nc.scalar.activation(wu8_t, wu_t, copyf, scale=SW)

                    wg_v = wg_t.rearrange("p (k i) -> p k i", i=P)
                    wu_v = wu_t.rearrange("p (k i) -> p k i", i=P)
                    wg8_v = wg8_t.rearrange("p (a b i) -> p a b i", a=KHP, b=2)
                    wu8_v = wu8_t.rearrange("p (a b i) -> p a b i", a=KHP, b=2)

                    psum_g = psum_pool.tile([P, W16[s]], F32, tag="g")
                    psum_u = psum_pool.tile([P, W16[s]], F32, tag="u")
                    psum_g8 = psum_pool.tile([P, W8[s]], F32, tag="g8")
                    psum_u8 = psum_pool.tile([P, W8[s]], F32, tag="u8")
                    for k in range(KH):
                        nc.tensor.matmul(
                            psum_g, wg_v[:, k, :], xsb[s][:, k, :],
                            start=(k == 0), stop=(k == KH - 1),
                        )
                    for k in range(KH):
                        nc.tensor.matmul(
                            psum_u, wu_v[:, k, :], xsb[s][:, k, :],
                            start=(k == 0), stop=(k == KH - 1),
                        )
                    for c0, c1 in _col_chunks(W8[s]):
                        for kp in range(KHP):
                            nc.tensor.matmul(
                                psum_g8[:, c0:c1], wg8_v[:, kp, :, :],
                                xsb8[s][:, kp, :, c0:c1],
                                start=(kp == 0), stop=(kp == KHP - 1),
                                perf_mode=DR,
                            )
                    for c0, c1 in _col_chunks(W8[s]):
                        for kp in range(KHP):
                            nc.tensor.matmul(
                                psum_u8[:, c0:c1], wu8_v[:, kp, :, :],
                                xsb8[s][:, kp, :, c0:c1],
                                start=(kp == 0), stop=(kp == KHP - 1),
                                perf_mode=DR,
                            )
                    tg = tpool.tile([P, W16[s]], F32, tag="gelu")
                    nc.scalar.activation(tg, psum_g, gelu)
                    nc.vector.tensor_mul(hsb[s][:, m, :], tg, psum_u)
                    tg8 = tpool.tile([P, W8[s]], F32, tag="gelu8")
                    nc.scalar.activation(tg8, psum_g8, gelu, scale=1.0 / SW)
                    nc.vector.tensor_mul(
                        hsb8[s][:, m // 2, m % 2, :], tg8, psum_u8
                    )

            # ---- Phase 2: downT partials -> yT (fp16); lo columns = 64*down
            ptags = ["g", "u"]
            p8tags = ["g8", "u8"]
            pidx = 0
            for m2 in range(MH):
                wd_ts = [
                    load_w(Wd_a[s], m2, "wd", pool=wdpool, cols=KI4 * P)
                    for s in range(NS)
                ]
                wd8_ts = []
                for s in range(NS):
                    wd8_t = wd8pool.tile([P, KI4 * P], F8, tag="wd8")
                    nc.scalar.activation(wd8_t, wd_ts[s], copyf, scale=SW)
                    wd8_ts.append(wd8_t)

                ysb = tpool.tile([P, CT], F16, tag="ystage", bufs=2)
                for s in range(NS):
                    wd_v = wd_ts[s].rearrange("p (k i) -> p k i", i=P)
                    psum_d = psum_pool.tile([P, W16[s]], F32, tag=ptags[pidx % 2])
                    for k2 in range(KI4):
                        nc.tensor.matmul(
                            psum_d, wd_v[:, k2, :], hsb[s][:, k2, :],
                            start=(k2 == 0), stop=(k2 == KI4 - 1),
                        )
                    nc.vector.tensor_copy(
                        ysb[:, o16[s] : o16[s] + W16[s]], psum_d
                    )
                    pidx += 1
                for s in range(NS):
                    wd8_v = wd8_ts[s].rearrange(
                        "p (a b i) -> p a b i", a=KI4P, b=2
                    )
                    psum_d8 = psum_pool.tile(
                        [P, W8[s]], F32, tag=p8tags[pidx % 2]
                    )
                    for c0, c1 in _col_chunks(W8[s]):
                        for kp in range(KI4P):
                            nc.tensor.matmul(
                                psum_d8[:, c0:c1], wd8_v[:, kp, :, :],
                                hsb8[s][:, kp, :, c0:c1],
                                start=(kp == 0), stop=(kp == KI4P - 1),
                                perf_mode=DR,
                            )
                    nc.vector.tensor_copy(
                        ysb[:, o8[s] : o8[s] + W8[s]], psum_d8
                    )
                    pidx += 1
                    if m2 == MH - 1 and s == NS - 2:
                        # flush everything but the last fp8 slice early
                        nc.sync.dma_start(
                            out=yT_r[:, m2, 0 : o8[NS - 1]],
                            in_=ysb[:, 0 : o8[NS - 1]],
                        )
                if m2 < MH - 1:
                    nc.sync.dma_start(out=yT_r[:, m2, :], in_=ysb)
                else:
                    nc.sync.dma_start(
                        out=yT_r[:, m2, o8[NS - 1] : CT],
                        in_=ysb[:, o8[NS - 1] : CT],
                    )

    nc.compile()
    return nc


def _get_program_quad_mix(W16, W8) -> "bass.Bass":
    key = ("quadmix", tuple(W16), tuple(W8))
    if key not in _PROGRAM_CACHE:
        _PROGRAM_CACHE[key] = _build_program_quad_mix(W16, W8)
    return _PROGRAM_CACHE[key]


def _prep_w_gu_q(w):  # [H, IQ] f32 -> [MI4, P, KH*P] fp16
    return np.ascontiguousarray(
        w.astype(np.float16).reshape(KH, P, MI4, P).transpose(2, 1, 0, 3)
    ).reshape(MI4, P, KH * P)


def _prep_w_d_q(w):  # [IQ, H] f32 -> [MH, P, KI4*P] fp16
    return np.ascontiguousarray(
        w.astype(np.float16).reshape(KI4, P, MH, P).transpose(2, 1, 0, 3)
    ).reshape(MH, P, KI4 * P)


def _ceil8(n):
    return max(8, -(-n // 8) * 8)


def kernel(x, selected_experts, routing_weights, Wg, Wu, Wd):
    global LAST_RESULTS
    x = np.asarray(x, dtype=np.float32)
    se = np.asarray(selected_experts).astype(np.int64)
    rw = np.asarray(routing_weights).astype(np.float32)
    Wg = np.asarray(Wg, dtype=np.float32)
    Wu = np.asarray(Wu, dtype=np.float32)
    Wd = np.asarray(Wd, dtype=np.float32)

    T, K = se.shape
    assert x.shape == (T, H) and Wg.shape == (E, H, I) and Wd.shape == (E, I, H)

    # Dense route matrix, identical to the reference's scatter-add (merges
    # duplicate expert picks within a token by summing their weights).
    flat_t = np.repeat(np.arange(T), K)
    flat_e = se.ravel()
    route = np.zeros((T, E), np.float32)
    np.add.at(route, (flat_t, flat_e), rw.ravel())
    present = np.zeros((T, E), bool)
    present[flat_t, flat_e] = True

    idx_lists = [np.nonzero(present[:, e])[0] for e in range(E)]
    counts = np.array([len(ix) for ix in idx_lists])

    # Per-expert class split by route weight; lo pairs sorted by descending
    # weight so the head of each lo list promotes cleanly into hi padding.
    ix16_l, ix8_l = [], []
    for e in range(E):
        ix = idx_lists[e]
        r = route[ix, e]
        lo = r < THETA
        ix16_l.append(ix[~lo])
        lo_ix = ix[lo]
        lo_ix = lo_ix[np.argsort(-route[lo_ix, e], kind="stable")]
        ix8_l.append(lo_ix)
    c16 = [len(v) for v in ix16_l]
    c8 = [len(v) for v in ix8_l]

    # Optimal pairing of experts into 4 slices (one expert per group):
    # cost = 384*W16 + 192*W8 per slice, promotion-aware.
    def pair_cost(i, j):
        w16 = _ceil8(max(c16[i], c16[j]))
        c8i = c8[i] - min(w16 - c16[i], c8[i])
        c8j = c8[j] - min(w16 - c16[j], c8[j])
        return 384 * w16 + 192 * _ceil8(max(c8i, c8j))

    best = [None]

    def match(rem, acc, cost):
        if not rem:
            if best[0] is None or cost < best[0][0]:
                best[0] = (cost, list(acc))
            return
        a = rem[0]
        for k in range(1, len(rem)):
            b = rem[k]
            acc.append((a, b))
            match(rem[1:k] + rem[k + 1 :], acc, cost + pair_cost(a, b))
            acc.pop()

    match(list(range(E)), [], 0)
    pairs = best[0][1]
    # widest slices first (their x streams earliest are... keep determinism)
    pairs.sort(key=lambda p: -max(c16[p[0]], c16[p[1]]))
    groups = [np.array([p[0] for p in pairs]), np.array([p[1] for p in pairs])]

    # Promote top lo pairs into hi padding, then size the lo widths.
    W16, W8 = [], []
    for s, (i, j) in enumerate(pairs):
        w16 = _ceil8(max(c16[i], c16[j]))
        W16.append(w16)
        for e in (i, j):
            np_e = min(w16 - c16[e], c8[e])
            if np_e > 0:
                ix16_l[e] = np.concatenate([ix16_l[e], ix8_l[e][:np_e]])
                ix8_l[e] = ix8_l[e][np_e:]
        W8.append(_ceil8(max(len(ix8_l[i]), len(ix8_l[j]))))

    if max(W16) <= 512 and max(W8) <= 512:
        return _kernel_quad_mix(
            x, route, ix16_l, ix8_l, groups, W16, W8, Wg, Wu, Wd
        )
    return _kernel_single(x, route, present, idx_lists, Wg, Wu, Wd)


def _kernel_quad_mix(x, route, ix16_l, ix8_l, groups, W16, W8, Wg, Wu, Wd):
    global LAST_RESULTS
    T = x.shape[0]
    o16 = np.concatenate([[0], np.cumsum(W16)])
    CT16 = int(o16[-1])
    o8 = np.concatenate([[CT16], CT16 + np.cumsum(W8)])

    nc = _get_program_quad_mix(W16, W8)
    xhalf = x.astype(np.float16)

    def pack_x16(ix, C, s):
        xt = np.zeros((H, C), np.float16)
        if len(ix):
            xt[:, : len(ix)] = xhalf[ix].T
        if s == 0:
            # [H, C] -> [XG=4, P, 4*C]: partition-major 4-chunk groups
            return np.ascontiguousarray(
                xt.reshape(4, 4, P, C).transpose(0, 2, 1, 3)
            ).reshape(4, P, 4 * C)
        # flat partition-major: [P, KH*C], one DMA trigger
        return np.ascontiguousarray(
            xt.reshape(KH, P, C).transpose(1, 0, 2)
        ).reshape(P, KH * C)

    def pack_x8(ix, C):
        # [H, C] fp8 -> [P, KHP*2*C]: (p, kp, i, c), k = kp*256 + i*128 + p
        xt = np.zeros((H, C), np.float32)
        if len(ix):
            xt[:, : len(ix)] = x[ix].T
        x8 = xt.astype(FP8NP)
        return np.ascontiguousarray(
            x8.reshape(KHP, 2, P, C).transpose(2, 0, 1, 3)
        ).reshape(P, KHP * 2 * C)

    in_maps = []
    for g in range(2):
        exps = [int(e) for e in groups[g]]
        xpacks = {}
        for s in range(NS):
            xpacks[f"xT{s}"] = pack_x16(ix16_l[exps[s]], W16[s], s)
            xpacks[f"xT8_{s}"] = pack_x8(ix8_l[exps[s]], W8[s])
        for q in range(4):
            sl = slice(q * IQ, (q + 1) * IQ)
            m = dict(xpacks)
            for s in range(NS):
                m[f"Wg{s}"] = _prep_w_gu_q(Wg[exps[s]][:, sl])
                m[f"Wu{s}"] = _prep_w_gu_q(Wu[exps[s]][:, sl])
                m[f"Wd{s}"] = _prep_w_d_q(Wd[exps[s]][sl, :])
            in_maps.append(m)
    res = run_bass_kernel_spmd(nc, in_maps, core_ids=list(range(E)))
    LAST_RESULTS = res

    out = np.zeros((T, H), np.float32)
    for g in range(2):
        exps = [int(e) for e in groups[g]]
        ysum = sum(
            res.results[4 * g + q]["yT"].astype(np.float32) for q in range(4)
        )  # [H, CT]
        for s in range(NS):
            ix = ix16_l[exps[s]]
            if len(ix):
                out[ix] += (
                    route[ix, exps[s]][:, None]
                    * ysum[:, o16[s] : o16[s] + len(ix)].T
                )
            ix = ix8_l[exps[s]]
            if len(ix):
                out[ix] += (
                    (route[ix, exps[s]] / YS8)[:, None]
                    * ysum[:, o8[s] : o8[s] + len(ix)].T
                )
    return out


def _build_program_single(C: int) -> "bass.Bass":
    """Fallback: one core = one expert's MLP on C tokens (fp16 only)."""
    assert C % 8 == 0 and 256 <= C <= 512

    nc = bacc.Bacc("TRN2", target_bir_lowering=False)

    XG = KH // 4
    xT = nc.dram_tensor("xT", [XG, P, 4 * C], F16, kind="ExternalInput")
    Wg = nc.dram_tensor("Wg", [MI, P, KH * P], F16, kind="ExternalInput")
    Wu = nc.dram_tensor("Wu", [MI, P, KH * P], F16, kind="ExternalInput")
    Wd = nc.dram_tensor("Wd", [MH, P, KI * P], F16, kind="ExternalInput")
    yT = nc.dram_tensor("yT", [H, C], F32, kind="ExternalOutput")

    xT_r = xT.ap()  # [XG, 128, 4*C]
    yT_r = yT.rearrange("(m p) c -> p m c", p=P)  # [128, 16, C]
    Wg_a, Wu_a, Wd_a = Wg.ap(), Wu.ap(), Wd.ap()

    gelu = mybir.ActivationFunctionType.Gelu_apprx_tanh

    with tile.TileContext(nc) as tc:
        with (
            tc.tile_pool(name="xpool", bufs=1) as xpool,
            tc.tile_pool(name="hpool", bufs=1) as hpool,
            tc.tile_pool(name="wpool", bufs=6) as wpool,
            tc.tile_pool(name="tpool", bufs=3) as tpool,
            tc.tile_pool(name="warm", bufs=1) as warm_pool,
            tc.tile_pool(name="psum2", bufs=2, space="PSUM") as psum_pool,
            tc.tile_pool(name="psumw", bufs=1, space="PSUM") as psum_warm,
        ):
            wz = warm_pool.tile([P, P], F16)
            xz = warm_pool.tile([P, C], F16)
            nc.vector.memset(wz, 0.0)
            nc.vector.memset(xz, 0.0)
            psum_w = psum_warm.tile([P, C], F32, tag="warm")
            for _ in range(13):
                nc.tensor.matmul(psum_w, wz, xz, start=True, stop=True)

            xsb = xpool.tile([P, KH, C], F16)
            hsb = hpool.tile([P, KI, C], F16)

            def load_w(dram_ap, t, tag, splits=1):
                wt = wpool.tile([P, KH * P], F16, tag=tag, name=f"w_{tag}_{t}")
                step = (KH * P) // splits
                for s in range(splits):
                    nc.sync.dma_start(
                        out=wt[:, s * step : (s + 1) * step],
                        in_=dram_ap[t, :, s * step : (s + 1) * step],
                    )
                return wt.rearrange("p (k i) -> p k i", i=P)

            def load_x(g):
                nc.sync.dma_start(out=xsb[:, 4 * g : 4 * (g + 1), :], in_=xT_r[g])

            wg_t0 = load_w(Wg_a, 0, "wg", splits=2)
            load_x(0)
            load_x(1)
            wu_t0 = load_w(Wu_a, 0, "wu", splits=2)
            load_x(2)
            load_x(3)
            wg_t1 = load_w(Wg_a, 1, "wg")
            wu_t1 = load_w(Wu_a, 1, "wu")

            for m in range(MI):
                if m == 0:
                    wg_t, wu_t = wg_t0, wu_t0
                elif m == 1:
                    wg_t, wu_t = wg_t1, wu_t1
                else:
                    wg_t = load_w(Wg_a, m, "wg")
                    wu_t = load_w(Wu_a, m, "wu")

                psum_g = psum_pool.tile([P, C], F32, tag="g")
                psum_u = psum_pool.tile([P, C], F32, tag="u")
                for k in range(KH):
                    nc.tensor.matmul(
                        psum_g, wg_t[:, k, :], xsb[:, k, :],
                        start=(k == 0), stop=(k == KH - 1),
                    )
                for k in range(KH):
                    nc.tensor.matmul(
                        psum_u, wu_t[:, k, :], xsb[:, k, :],
                        start=(k == 0), stop=(k == KH - 1),
                    )
                tg = tpool.tile([P, C], F32, tag="gelu")
                nc.scalar.activation(tg, psum_g, gelu)
                nc.vector.tensor_mul(hsb[:, m, :], tg, psum_u)

            for m2 in range(MH):
                wd_t = wpool.tile([P, KI * P], F16, tag="wd", name=f"w_wd_{m2}")
                nc.sync.dma_start(out=wd_t, in_=Wd_a[m2])
                wd_v = wd_t.rearrange("p (k i) -> p k i", i=P)
                if m2 < MH - 2:
                    pieces = [(0, C)]
                elif m2 == MH - 2:
                    pieces = [(0, C // 2), (C // 2, C)]
                else:
                    q = -(-C // 32) * 8
                    pieces = [(i * q, min((i + 1) * q, C)) for i in range(4)]
                    pieces = [(a, b) for a, b in pieces if b > a]
                ptags = ["d", "g"]
                for pi, (c0, c1) in enumerate(pieces):
                    psum_d = psum_pool.tile([P, c1 - c0], F32, tag=ptags[pi % 2])
                    for k2 in range(KI):
                        nc.tensor.matmul(
                            psum_d, wd_v[:, k2, :], hsb[:, k2, c0:c1],
                            start=(k2 == 0), stop=(k2 == KI - 1),
                        )
                    ysb = tpool.tile([P, c1 - c0], F32, tag="y")
                    nc.vector.tensor_copy(ysb, psum_d)
                    nc.sync.dma_start(out=yT_r[:, m2, c0:c1], in_=ysb)

    nc.compile()
    return nc


def _prep_w_gu(w):  # [H, I] f32 -> [MI, P, KH*P] fp16
    return np.ascontiguousarray(
        w.astype(np.float16).reshape(KH, P, MI, P).transpose(2, 1, 0, 3)
    ).reshape(MI, P, KH * P)


def _prep_w_d(w):  # [I, H] f32 -> [MH, P, KI*P] fp16
    return np.ascontiguousarray(
        w.astype(np.float16).reshape(KI, P, MH, P).transpose(2, 1, 0, 3)
    ).reshape(MH, P, KI * P)


def _get_program_single(C: int) -> "bass.Bass":
    key = ("single", C)
    if key not in _PROGRAM_CACHE:
        _PROGRAM_CACHE[key] = _build_program_single(C)
    return _PROGRAM_CACHE[key]


def _kernel_single(x, route, present, idx_lists, Wg, Wu, Wd):
    """Fallback: one expert per core, capacity-chunked (handles any skew)."""
    global LAST_RESULTS
    T = x.shape[0]
    chunked = [
        [ix[s : s + 512] for s in range(0, max(len(ix), 1), 512)]
        for ix in idx_lists
    ]
    n_pass = max(len(ch) for ch in chunked)

    out = np.zeros((T, H), np.float32)
    for p in range(n_pass):
        parts = [ch[p] if p < len(ch) else np.empty(0, np.int64) for ch in chunked]
        max_count = max(len(ix) for ix in parts)
        C = max(256, min(512, -(-max(max_count, 1) // 8) * 8))
        nc = _get_program_single(C)
        in_maps = []
        for e in range(E):
            ix = parts[e]
            xT_e = np.zeros((H, C), np.float16)
            if len(ix):
                xT_e[:, : len(ix)] = x[ix].T.astype(np.float16)
            xT_e = np.ascontiguousarray(
                xT_e.reshape(4, 4, P, C).transpose(0, 2, 1, 3)
            ).reshape(4, P, 4 * C)
            in_maps.append(
                {
                    "xT": xT_e,
                    "Wg": _prep_w_gu(Wg[e]),
                    "Wu": _prep_w_gu(Wu[e]),
                    "Wd": _prep_w_d(Wd[e]),
                }
            )
        res = run_bass_kernel_spmd(nc, in_maps, core_ids=list(range(E)))
        LAST_RESULTS = res
        for e in range(E):
            ix = parts[e]
            if len(ix) == 0:
                continue
            yT_e = res.results[e]["yT"]  # [H, C]
            out[ix] += route[ix, e][:, None] * yT_e[:, : len(ix)].T
    return out


# revision 20
# speedup vs baseline: 5.3437x; 1.0116x over previous
"""MoE (Gemma-style 8-expert top-2) Trainium2 kernel — expert-quad / I-quarter,
mixed fp16/fp8 precision classes split by routing weight.

Strategy (8 NeuronCores):
  - Host: merge duplicate (token, expert) picks, build per-expert token lists.
    Split each expert's tokens into a hi class (route weight >= 0.4, fp16
    pipeline) and a lo class (fp8 e4m3 pipeline at 2x PE rate via DoubleRow
    matmuls).  Output error is dominated by the fp8 class but scales as
    theta^1.5 of the route-weight quantile -> ~1.5e-2 rel err at theta=0.4
    (measured on the reference data; threshold 2e-2).
  - Sort experts by count desc; group A = ranks {0,2,4,6}, group B =
    {1,3,5,7}.  Cores 0-3 serve group A (one I-quarter each), cores 4-7
    group B.  Slice widths W16[s]/W8[s] = ceil8(max over the two groups).
  - Device (per core): weights stream in fp16 once; fp8 copies are derived
    on-chip (x8 = fp8 scale 1; W8 = fp8(8*W16); h8 = fp8(8*h); psum_d8 =
    64*down, un-scaled on the host).  Phase 1 slice-serial; phase 2 emits
    fp16 partials, lo-class columns carry the 64x scale.
  - Host: combine with route weights (lo columns divided by 64).

Falls back to fp16-only programs when slice widths exceed limits.
"""

import numpy as np
import ml_dtypes

import concourse.bass as bass
import concourse.mybir as mybir
import concourse.tile as tile
from concourse import bacc


def _install_ntff_hook_shim():
    import sys
    import types

    try:
        import antenv

        try:
            from antenv import axon_hooks  # noqa: F401

            return
        except ImportError:
            pass
        mod = types.ModuleType("antenv.axon_hooks")
        mod._hook = None
        mod.set_axon_ntff_profile_hook = lambda h: setattr(mod, "_hook", h)
        mod.get_axon_ntff_profile_hook = lambda: mod._hook
        sys.modules["antenv.axon_hooks"] = mod
        antenv.axon_hooks = mod
        import os

        so_path = "/opt/axon/libaxon_pjrt.so"
        if os.path.exists(so_path):
            from trn_agent_boot.trn_boot import _ntff_profile_via_ctypes

            mod._hook = _ntff_profile_via_ctypes(so_path)
    except Exception:
        pass


_install_ntff_hook_shim()

from concourse.bass_utils import run_bass_kernel_spmd

H = 2048
I = 4096
E = 8
P = 128
KH = H // P  # 16 contraction chunks for gate/up
KHP = KH // 2  # 8 DoubleRow chunk-pairs
IQ = I // 4  # per-core I slice (1024)
MI4 = IQ // P  # 8 output tiles of I/4
KI4 = IQ // P  # 8 contraction chunks for down (per I-quarter)
KI4P = KI4 // 2  # 4 DoubleRow chunk-pairs for down
MH = H // P  # 16 output tiles of H
MI = I // P  # 32 output tiles of I (fallback single-expert program)
KI = I // P  # 32 contraction chunks for down (fallback)
NS = 4  # slices (experts) per core
F32 = mybir.dt.float32
F16 = mybir.dt.float16
F8 = mybir.dt.float8e4
FP8NP = ml_dtypes.float8_e4m3
DR = mybir.MatmulPerfMode.DoubleRow

THETA = 0.48  # route-weight cutoff: below -> fp8 class
SW = 8.0  # fp8 weight scale (dodges e4m3 subnormal floor)
YS8 = 64.0  # lo-class output carries SW*SW scale

LAST_RESULTS = None

_PROGRAM_CACHE: dict[tuple, "bass.Bass"] = {}


def _col_chunks(w):
    """Column chunks <=256 wide (DoubleRow moving free dim limit is 512)."""
    if w <= 256:
        return [(0, w)]
    half = -(-(w // 2) // 8) * 8
    return [(0, half), (half, w)]


def _build_program_quad_mix(W16, W8) -> "bass.Bass":
    """Bass program for one core: I/4 slice of a 4-expert MLP, 2 precision
    classes per slice.  Slice s has W16[s] fp16 columns and W8[s] fp8
    columns; y columns are [all fp16 slices | all fp8 slices]."""
    W16 = tuple(W16)
    W8 = tuple(W8)
    assert all(w % 8 == 0 and 8 <= w <= 512 for w in W16 + W8)
    o16 = [0]
    for w in W16:
        o16.append(o16[-1] + w)
    CT16 = o16[-1]
    o8 = [CT16]
    for w in W8:
        o8.append(o8[-1] + w)
    CT = o8[-1]

    nc = bacc.Bacc("TRN2", target_bir_lowering=False)

    XG = KH // 4
    # Slice 0's x streams in the early window: 4-chunk layout so the first
    # k-chunks' semaphore fires before the whole slab lands.  Later slices
    # prefetch during compute: flat partition-major, one trigger each.
    xTs = [
        nc.dram_tensor(
            f"xT{s}",
            [2, P, 8 * W16[s]] if s == 0 else [P, KH * W16[s]],
            F16,
            kind="ExternalInput",
        )
        for s in range(NS)
    ]
    xT8s = [
        nc.dram_tensor(f"xT8_{s}", [P, KHP * 2 * W8[s]], F8, kind="ExternalInput")
        for s in range(NS)
    ]
    Wgs = [
        nc.dram_tensor(f"Wg{s}", [MI4, P, KH * P], F16, kind="ExternalInput")
        for s in range(NS)
    ]
    Wus = [
        nc.dram_tensor(f"Wu{s}", [MI4, P, KH * P], F16, kind="ExternalInput")
        for s in range(NS)
    ]
    Wds = [
        nc.dram_tensor(f"Wd{s}", [MH, P, KI4 * P], F16, kind="ExternalInput")
        for s in range(NS)
    ]
    yT = nc.dram_tensor("yT", [H, CT], F16, kind="ExternalOutput")

    xT_r = [t.ap() for t in xTs]
    xT8_r = [t.ap() for t in xT8s]
    Wg_a = [t.ap() for t in Wgs]
    Wu_a = [t.ap() for t in Wus]
    Wd_a = [t.ap() for t in Wds]
    yT_r = yT.rearrange("(m p) c -> p m c", p=P)  # [128, 16, CT]

    gelu = mybir.ActivationFunctionType.Gelu_apprx_tanh
    copyf = mybir.ActivationFunctionType.Copy

    with tile.TileContext(nc) as tc:
        with (
            tc.tile_pool(name="xpool", bufs=1) as xpool,
            tc.tile_pool(name="hpool", bufs=1) as hpool,
            tc.tile_pool(name="wpool", bufs=3) as wpool,
            tc.tile_pool(name="w8pool", bufs=2) as w8pool,
            tc.tile_pool(name="wdpool", bufs=12) as wdpool,
            tc.tile_pool(name="wd8pool", bufs=6) as wd8pool,
            tc.tile_pool(name="tpool", bufs=3) as tpool,
            tc.tile_pool(name="warm", bufs=1) as warm_pool,
            tc.tile_pool(name="psum2", bufs=2, space="PSUM") as psum_pool,
        ):
            # --- PE warm-up: dummy matmuls cover BOTH the ~3.4us HAM clock
            # ramp AND the ~6us until the first weight/x bytes arrive.
            wz = warm_pool.tile([P, P], F16)
            xz = warm_pool.tile([P, W16[0]], F16)
            nc.vector.memset(wz, 0.0)
            nc.vector.memset(xz, 0.0)
            psum_w = psum_pool.tile([P, W16[0]], F32, tag="g")
            for _ in range(20):
                nc.tensor.matmul(psum_w, wz, xz, start=True, stop=True)

            xsb = [
                xpool.tile([P, KH, W16[s]], F16, tag=f"x{s}", name=f"xsb{s}")
                for s in range(NS)
            ]
            xsb8 = [
                xpool.tile([P, KHP, 2, W8[s]], F8, tag=f"x8{s}", name=f"xsb8{s}")
                for s in range(NS)
            ]
            hsb = [
                hpool.tile([P, KI4, W16[s]], F16, tag=f"h{s}", name=f"hsb{s}")
                for s in range(NS)
            ]
            hsb8 = [
                hpool.tile([P, KI4P, 2, W8[s]], F8, tag=f"h8{s}", name=f"hsb8{s}")
                for s in range(NS)
            ]

            def load_w(dram_ap, t, tag, pool=wpool, cols=KH * P, splits=1):
                wt = pool.tile([P, cols], F16, tag=tag, name=f"w_{tag}_{t}")
                step = cols // splits
                for sp in range(splits):
                    nc.sync.dma_start(
                        out=wt[:, sp * step : (sp + 1) * step],
                        in_=dram_ap[t, :, sp * step : (sp + 1) * step],
                    )
                return wt

            def load_x(s):
                if s == 0:
                    # two triggers: halves (fewer ~650ns trigger issues in
                    # the early window, first-half semaphore still early)
                    for g2 in range(2):
                        nc.sync.dma_start(
                            out=xsb[s][:, 8 * g2 : 8 * (g2 + 1), :],
                            in_=xT_r[s][g2],
                        )
                else:
                    nc.sync.dma_start(
                        out=xsb[s].rearrange("p a b -> p (a b)"), in_=xT_r[s]
                    )

            def load_x8(s):
                nc.sync.dma_start(
                    out=xsb8[s].rearrange("p a b c -> p (a b c)"), in_=xT8_r[s]
                )

            # Early DMA order: first pass's m0 weights + its x only.
            s0 = 0
            wg_t0 = load_w(Wg_a[s0], 0, "wg")
            load_x(s0)
            load_x8(s0)
            wu_t0 = load_w(Wu_a[s0], 0, "wu")
            wg_t1 = load_w(Wg_a[s0], 1, "wg")
            wu_t1 = load_w(Wu_a[s0], 1, "wu")

            # ---- Phase 1: gateT/upT -> hT (both classes), slice-serial
            for si in range(NS):
                s = si
                for m in range(MI4):
                    if si == 0 and m == 0:
                        wg_t, wu_t = wg_t0, wu_t0
                    elif si == 0 and m == 1:
                        wg_t, wu_t = wg_t1, wu_t1
                    else:
                        wg_t = load_w(Wg_a[s], m, "wg")
                        wu_t = load_w(Wu_a[s], m, "wu")
                    if m == 2 and si < NS - 1:
                        load_x(si + 1)
                        load_x8(si + 1)

                    # on-chip fp8 copies of this m-tile's weights (x SW).
                    # Scalar engine only: DVE/GpSimd tensor ops collapse
                    # ~15x under PE+DMA SBUF load; scalar ACTIVATE stays at
                    # 1 elem/lane/cycle.
                    wg8_t = w8pool.tile([P, KH * P], F8, tag="wg8")
                    wu8_t = w8pool.tile([P, KH * P], F8, tag="wu8")
                    nc.scalar.activation(wg8_t, wg_t, copyf, scale=SW)
                    nc.scalar.activation(wu8_t, wu_t, copyf, scale=SW)

                    wg_v = wg_t.rearrange("p (k i) -> p k i", i=P)
                    wu_v = wu_t.rearrange("p (k i) -> p k i", i=P)
                    wg8_v = wg8_t.rearrange("p (a b i) -> p a b i", a=KHP, b=2)
                    wu8_v = wu8_t.rearrange("p (a b i) -> p a b i", a=KHP, b=2)

                    psum_g = psum_pool.tile([P, W16[s]], F32, tag="g")
                    psum_u = psum_pool.tile([P, W16[s]], F32, tag="u")
                    psum_g8 = psum_pool.tile([P, W8[s]], F32, tag="g8")
                    psum_u8 = psum_pool.tile([P, W8[s]], F32, tag="u8")
                    for k in range(KH):
                        nc.tensor.matmul(
                            psum_g, wg_v[:, k, :], xsb[s][:, k, :],
                            start=(k == 0), stop=(k == KH - 1),
                        )
                    for k in range(KH):
                        nc.tensor.matmul(
                            psum_u, wu_v[:, k, :], xsb[s][:, k, :],
                            start=(k == 0), stop=(k == KH - 1),
                        )
                    for c0, c1 in _col_chunks(W8[s]):
                        for kp in range(KHP):
                            nc.tensor.matmul(
                                psum_g8[:, c0:c1], wg8_v[:, kp, :, :],
                                xsb8[s][:, kp, :, c0:c1],
                                start=(kp == 0), stop=(kp == KHP - 1),
                                perf_mode=DR,
                            )
                    for c0, c1 in _col_chunks(W8[s]):
                        for kp in range(KHP):
                            nc.tensor.matmul(
                                psum_u8[:, c0:c1], wu8_v[:, kp, :, :],
                                xsb8[s][:, kp, :, c0:c1],
                                start=(kp == 0), stop=(kp == KHP - 1),
                                perf_mode=DR,
                            )
                    tg = tpool.tile([P, W16[s]], F32, tag="gelu")
                    nc.scalar.activation(tg, psum_g, gelu)
                    nc.vector.tensor_mul(hsb[s][:, m, :], tg, psum_u)
                    tg8 = tpool.tile([P, W8[s]], F32, tag="gelu8")
                    nc.scalar.activation(tg8, psum_g8, gelu, scale=1.0 / SW)
                    nc.vector.tensor_mul(
                        hsb8[s][:, m // 2, m % 2, :], tg8, psum_u8
                    )

            # ---- Phase 2: downT partials -> yT (fp16); lo columns = 64*down
            ptags = ["g", "u"]
            p8tags = ["g8", "u8"]
            pidx = 0
            for m2 in range(MH):
                wd_ts = [
                    load_w(Wd_a[s], m2, "wd", pool=wdpool, cols=KI4 * P)
                    for s in range(NS)
                ]
                wd8_ts = []
                for s in range(NS):
                    wd8_t = wd8pool.tile([P, KI4 * P], F8, tag="wd8")
                    nc.scalar.activation(wd8_t, wd_ts[s], copyf, scale=SW)
                    wd8_ts.append(wd8_t)

                ysb = tpool.tile([P, CT], F16, tag="ystage", bufs=2)
                for s in range(NS):
                    wd_v = wd_ts[s].rearrange("p (k i) -> p k i", i=P)
                    psum_d = psum_pool.tile([P, W16[s]], F32, tag=ptags[pidx % 2])
                    for k2 in range(KI4):
                        nc.tensor.matmul(
                            psum_d, wd_v[:, k2, :], hsb[s][:, k2, :],
                            start=(k2 == 0), stop=(k2 == KI4 - 1),
                        )
                    nc.vector.tensor_copy(
                        ysb[:, o16[s] : o16[s] + W16[s]], psum_d
                    )
                    pidx += 1
                last = m2 == MH - 1
                half = -(-(W8[NS - 1] // 2) // 8) * 8  # tail split point
                for s in range(NS):
                    wd8_v = wd8_ts[s].rearrange(
                        "p (a b i) -> p a b i", a=KI4P, b=2
                    )
                    if last and s == NS - 1:
                        pieces = [(0, half), (half, W8[s])]
                    else:
                        pieces = [(0, W8[s])]
                    for p0, p1 in pieces:
                        psum_d8 = psum_pool.tile(
                            [P, p1 - p0], F32, tag=p8tags[pidx % 2]
                        )
                        for c0, c1 in _col_chunks(p1 - p0):
                            for kp in range(KI4P):
                                nc.tensor.matmul(
                                    psum_d8[:, c0:c1], wd8_v[:, kp, :, :],
                                    hsb8[s][:, kp, :, p0 + c0 : p0 + c1],
                                    start=(kp == 0), stop=(kp == KI4P - 1),
                                    perf_mode=DR,
                                )
                        nc.vector.tensor_copy(
                            ysb[:, o8[s] + p0 : o8[s] + p1], psum_d8
                        )
                        pidx += 1
                        if last and s == NS - 1 and p1 == half:
                            # flush all but the final half-slice early
                            nc.sync.dma_start(
                                out=yT_r[:, m2, 0 : o8[s] + half],
                                in_=ysb[:, 0 : o8[s] + half],
                            )
                if not last:
                    nc.sync.dma_start(out=yT_r[:, m2, :], in_=ysb)
                else:
                    nc.sync.dma_start(
                        out=yT_r[:, m2, o8[NS - 1] + half : CT],
                        in_=ysb[:, o8[NS - 1] + half : CT],
                    )

    nc.compile()
    return nc


def _get_program_quad_mix(W16, W8) -> "bass.Bass":
    key = ("quadmix", tuple(W16), tuple(W8))
    if key not in _PROGRAM_CACHE:
        _PROGRAM_CACHE[key] = _build_program_quad_mix(W16, W8)
    return _PROGRAM_CACHE[key]


def _prep_w_gu_q(w):  # [H, IQ] f32 -> [MI4, P, KH*P] fp16
    return np.ascontiguousarray(
        w.astype(np.float16).reshape(KH, P, MI4, P).transpose(2, 1, 0, 3)
    ).reshape(MI4, P, KH * P)


def _prep_w_d_q(w):  # [IQ, H] f32 -> [MH, P, KI4*P] fp16
    return np.ascontiguousarray(
        w.astype(np.float16).reshape(KI4, P, MH, P).transpose(2, 1, 0, 3)
    ).reshape(MH, P, KI4 * P)


def _ceil8(n):
    return max(8, -(-n // 8) * 8)


def kernel(x, selected_experts, routing_weights, Wg, Wu, Wd):
    global LAST_RESULTS
    x = np.asarray(x, dtype=np.float32)
    se = np.asarray(selected_experts).astype(np.int64)
    rw = np.asarray(routing_weights).astype(np.float32)
    Wg = np.asarray(Wg, dtype=np.float32)
    Wu = np.asarray(Wu, dtype=np.float32)
    Wd = np.asarray(Wd, dtype=np.float32)

    T, K = se.shape
    assert x.shape == (T, H) and Wg.shape == (E, H, I) and Wd.shape == (E, I, H)

    # Dense route matrix, identical to the reference's scatter-add (merges
    # duplicate expert picks within a token by summing their weights).
    flat_t = np.repeat(np.arange(T), K)
    flat_e = se.ravel()
    route = np.zeros((T, E), np.float32)
    np.add.at(route, (flat_t, flat_e), rw.ravel())
    present = np.zeros((T, E), bool)
    present[flat_t, flat_e] = True

    idx_lists = [np.nonzero(present[:, e])[0] for e in range(E)]
    counts = np.array([len(ix) for ix in idx_lists])

    # Per-expert class split by route weight; lo pairs sorted by descending
    # weight so the head of each lo list promotes cleanly into hi padding.
    ix16_l, ix8_l = [], []
    for e in range(E):
        ix = idx_lists[e]
        r = route[ix, e]
        lo = r < THETA
        ix16_l.append(ix[~lo])
        lo_ix = ix[lo]
        lo_ix = lo_ix[np.argsort(-route[lo_ix, e], kind="stable")]
        ix8_l.append(lo_ix)
    c16 = [len(v) for v in ix16_l]
    c8 = [len(v) for v in ix8_l]

    # Optimal pairing of experts into 4 slices (one expert per group):
    # cost = 384*W16 + 192*W8 per slice, promotion-aware.
    def pair_cost(i, j):
        w16 = _ceil8(max(c16[i], c16[j]))
        c8i = c8[i] - min(w16 - c16[i], c8[i])
        c8j = c8[j] - min(w16 - c16[j], c8[j])
        return 384 * w16 + 192 * _ceil8(max(c8i, c8j))

    best = [None]

    def match(rem, acc, cost):
        if not rem:
            if best[0] is None or cost < best[0][0]:
                best[0] = (cost, list(acc))
            return
        a = rem[0]
        for k in range(1, len(rem)):
            b = rem[k]
            acc.append((a, b))
            match(rem[1:k] + rem[k + 1 :], acc, cost + pair_cost(a, b))
            acc.pop()

    match(list(range(E)), [], 0)
    pairs = best[0][1]
    # widest slices first (their x streams earliest are... keep determinism)
    pairs.sort(key=lambda p: -max(c16[p[0]], c16[p[1]]))
    groups = [np.array([p[0] for p in pairs]), np.array([p[1] for p in pairs])]

    # Promote top lo pairs into hi padding, then size the lo widths.
    W16, W8 = [], []
    for s, (i, j) in enumerate(pairs):
        w16 = _ceil8(max(c16[i], c16[j]))
        W16.append(w16)
        for e in (i, j):
            np_e = min(w16 - c16[e], c8[e])
            if np_e > 0:
                ix16_l[e] = np.concatenate([ix16_l[e], ix8_l[e][:np_e]])
                ix8_l[e] = ix8_l[e][np_e:]
        W8.append(_ceil8(max(len(ix8_l[i]), len(ix8_l[j]))))

    if max(W16) <= 512 and max(W8) <= 512:
        return _kernel_quad_mix(
            x, route, ix16_l, ix8_l, groups, W16, W8, Wg, Wu, Wd
        )
    return _kernel_single(x, route, present, idx_lists, Wg, Wu, Wd)


def _kernel_quad_mix(x, route, ix16_l, ix8_l, groups, W16, W8, Wg, Wu, Wd):
    global LAST_RESULTS
    T = x.shape[0]
    o16 = np.concatenate([[0], np.cumsum(W16)])
    CT16 = int(o16[-1])
    o8 = np.concatenate([[CT16], CT16 + np.cumsum(W8)])

    nc = _get_program_quad_mix(W16, W8)
    xhalf = x.astype(np.float16)

    def pack_x16(ix, C, s):
        xt = np.zeros((H, C), np.float16)
        if len(ix):
            xt[:, : len(ix)] = xhalf[ix].T
        if s == 0:
            # [H, C] -> [2, P, 8*C]: two partition-major half-slabs
            return np.ascontiguousarray(
                xt.reshape(2, 8, P, C).transpose(0, 2, 1, 3)
            ).reshape(2, P, 8 * C)
        # flat partition-major: [P, KH*C], one DMA trigger
        return np.ascontiguousarray(
            xt.reshape(KH, P, C).transpose(1, 0, 2)
        ).reshape(P, KH * C)

    def pack_x8(ix, C):
        # [H, C] fp8 -> [P, KHP*2*C]: (p, kp, i, c), k = kp*256 + i*128 + p
        xt = np.zeros((H, C), np.float32)
        if len(ix):
            xt[:, : len(ix)] = x[ix].T
        x8 = xt.astype(FP8NP)
        return np.ascontiguousarray(
            x8.reshape(KHP, 2, P, C).transpose(2, 0, 1, 3)
        ).reshape(P, KHP * 2 * C)

    in_maps = []
    for g in range(2):
        exps = [int(e) for e in groups[g]]
        xpacks = {}
        for s in range(NS):
            xpacks[f"xT{s}"] = pack_x16(ix16_l[exps[s]], W16[s], s)
            xpacks[f"xT8_{s}"] = pack_x8(ix8_l[exps[s]], W8[s])
        for q in range(4):
            sl = slice(q * IQ, (q + 1) * IQ)
            m = dict(xpacks)
            for s in range(NS):
                m[f"Wg{s}"] = _prep_w_gu_q(Wg[exps[s]][:, sl])
                m[f"Wu{s}"] = _prep_w_gu_q(Wu[exps[s]][:, sl])
                m[f"Wd{s}"] = _prep_w_d_q(Wd[exps[s]][sl, :])
            in_maps.append(m)
    res = run_bass_kernel_spmd(nc, in_maps, core_ids=list(range(E)))
    LAST_RESULTS = res

    out = np.zeros((T, H), np.float32)
    for g in range(2):
        exps = [int(e) for e in groups[g]]
        ysum = sum(
            res.results[4 * g + q]["yT"].astype(np.float32) for q in range(4)
        )  # [H, CT]
        for s in range(NS):
            ix = ix16_l[exps[s]]
            if len(ix):
                out[ix] += (
                    route[ix, exps[s]][:, None]
                    * ysum[:, o16[s] : o16[s] + len(ix)].T
                )
            ix = ix8_l[exps[s]]
            if len(ix):
                out[ix] += (
                    (route[ix, exps[s]] / YS8)[:, None]
                    * ysum[:, o8[s] : o8[s] + len(ix)].T
                )
    return out


def _build_program_single(C: int) -> "bass.Bass":
    """Fallback: one core = one expert's MLP on C tokens (fp16 only)."""
    assert C % 8 == 0 and 256 <= C <= 512

    nc = bacc.Bacc("TRN2", target_bir_lowering=False)

    XG = KH // 4
    xT = nc.dram_tensor("xT", [XG, P, 4 * C], F16, kind="ExternalInput")
    Wg = nc.dram_tensor("Wg", [MI, P, KH * P], F16, kind="ExternalInput")
    Wu = nc.dram_tensor("Wu", [MI, P, KH * P], F16, kind="ExternalInput")
    Wd = nc.dram_tensor("Wd", [MH, P, KI * P], F16, kind="ExternalInput")
    yT = nc.dram_tensor("yT", [H, C], F32, kind="ExternalOutput")

    xT_r = xT.ap()  # [XG, 128, 4*C]
    yT_r = yT.rearrange("(m p) c -> p m c", p=P)  # [128, 16, C]
    Wg_a, Wu_a, Wd_a = Wg.ap(), Wu.ap(), Wd.ap()

    gelu = mybir.ActivationFunctionType.Gelu_apprx_tanh

    with tile.TileContext(nc) as tc:
        with (
            tc.tile_pool(name="xpool", bufs=1) as xpool,
            tc.tile_pool(name="hpool", bufs=1) as hpool,
            tc.tile_pool(name="wpool", bufs=6) as wpool,
            tc.tile_pool(name="tpool", bufs=3) as tpool,
            tc.tile_pool(name="warm", bufs=1) as warm_pool,
            tc.tile_pool(name="psum2", bufs=2, space="PSUM") as psum_pool,
            tc.tile_pool(name="psumw", bufs=1, space="PSUM") as psum_warm,
        ):
            wz = warm_pool.tile([P, P], F16)
            xz = warm_pool.tile([P, C], F16)
            nc.vector.memset(wz, 0.0)
            nc.vector.memset(xz, 0.0)
            psum_w = psum_warm.tile([P, C], F32, tag="warm")
            for _ in range(13):
                nc.tensor.matmul(psum_w, wz, xz, start=True, stop=True)

            xsb = xpool.tile([P, KH, C], F16)
            hsb = hpool.tile([P, KI, C], F16)

            def load_w(dram_ap, t, tag, splits=1):
                wt = wpool.tile([P, KH * P], F16, tag=tag, name=f"w_{tag}_{t}")
                step = (KH * P) // splits
                for s in range(splits):
                    nc.sync.dma_start(
                        out=wt[:, s * step : (s + 1) * step],
                        in_=dram_ap[t, :, s * step : (s + 1) * step],
                    )
                return wt.rearrange("p (k i) -> p k i", i=P)

            def load_x(g):
                nc.sync.dma_start(out=xsb[:, 4 * g : 4 * (g + 1), :], in_=xT_r[g])

            wg_t0 = load_w(Wg_a, 0, "wg", splits=2)
            load_x(0)
            load_x(1)
            wu_t0 = load_w(Wu_a, 0, "wu", splits=2)
            load_x(2)
            load_x(3)
            wg_t1 = load_w(Wg_a, 1, "wg")
            wu_t1 = load_w(Wu_a, 1, "wu")

            for m in range(MI):
                if m == 0:
                    wg_t, wu_t = wg_t0, wu_t0
                elif m == 1:
                    wg_t, wu_t = wg_t1, wu_t1
                else:
                    wg_t = load_w(Wg_a, m, "wg")
                    wu_t = load_w(Wu_a, m, "wu")

                psum_g = psum_pool.tile([P, C], F32, tag="g")
                psum_u = psum_pool.tile([P, C], F32, tag="u")
                for k in range(KH):
                    nc.tensor.matmul(
                        psum_g, wg_t[:, k, :], xsb[:, k, :],
                        start=(k == 0), stop=(k == KH - 1),
                    )
                for k in range(KH):
                    nc.tensor.matmul(
                        psum_u, wu_t[:, k, :], xsb[:, k, :],
                        start=(k == 0), stop=(k == KH - 1),
                    )
                tg = tpool.tile([P, C], F32, tag="gelu")
                nc.scalar.activation(tg, psum_g, gelu)
                nc.vector.tensor_mul(hsb[:, m, :], tg, psum_u)

            for m2 in range(MH):
                wd_t = wpool.tile([P, KI * P], F16, tag="wd", name=f"w_wd_{m2}")
                nc.sync.dma_start(out=wd_t, in_=Wd_a[m2])
                wd_v = wd_t.rearrange("p (k i) -> p k i", i=P)
                if m2 < MH - 2:
                    pieces = [(0, C)]
                elif m2 == MH - 2:
                    pieces = [(0, C // 2), (C // 2, C)]
                else:
                    q = -(-C // 32) * 8
                    pieces = [(i * q, min((i + 1) * q, C)) for i in range(4)]
                    pieces = [(a, b) for a, b in pieces if b > a]
                ptags = ["d", "g"]
                for pi, (c0, c1) in enumerate(pieces):
                    psum_d = psum_pool.tile([P, c1 - c0], F32, tag=ptags[pi % 2])
                    for k2 in range(KI):
                        nc.tensor.matmul(
                            psum_d, wd_v[:, k2, :], hsb[:, k2, c0:c1],
                            start=(k2 == 0), stop=(k2 == KI - 1),
                        )
                    ysb = tpool.tile([P, c1 - c0], F32, tag="y")
                    nc.vector.tensor_copy(ysb, psum_d)
                    nc.sync.dma_start(out=yT_r[:, m2, c0:c1], in_=ysb)

    nc.compile()
    return nc


def _prep_w_gu(w):  # [H, I] f32 -> [MI, P, KH*P] fp16
    return np.ascontiguousarray(
        w.astype(np.float16).reshape(KH, P, MI, P).transpose(2, 1, 0, 3)
    ).reshape(MI, P, KH * P)


def _prep_w_d(w):  # [I, H] f32 -> [MH, P, KI*P] fp16
    return np.ascontiguousarray(
        w.astype(np.float16).reshape(KI, P, MH, P).transpose(2, 1, 0, 3)
    ).reshape(MH, P, KI * P)


def _get_program_single(C: int) -> "bass.Bass":
    key = ("single", C)
    if key not in _PROGRAM_CACHE:
        _PROGRAM_CACHE[key] = _build_program_single(C)
    return _PROGRAM_CACHE[key]


def _kernel_single(x, route, present, idx_lists, Wg, Wu, Wd):
    """Fallback: one expert per core, capacity-chunked (handles any skew)."""
    global LAST_RESULTS
    T = x.shape[0]
    chunked = [
        [ix[s : s + 512] for s in range(0, max(len(ix), 1), 512)]
        for ix in idx_lists
    ]
    n_pass = max(len(ch) for ch in chunked)

    out = np.zeros((T, H), np.float32)
    for p in range(n_pass):
        parts = [ch[p] if p < len(ch) else np.empty(0, np.int64) for ch in chunked]
        max_count = max(len(ix) for ix in parts)
        C = max(256, min(512, -(-max(max_count, 1) // 8) * 8))
        nc = _get_program_single(C)
        in_maps = []
        for e in range(E):
            ix = parts[e]
            xT_e = np.zeros((H, C), np.float16)
            if len(ix):
                xT_e[:, : len(ix)] = x[ix].T.astype(np.float16)
            xT_e = np.ascontiguousarray(
                xT_e.reshape(4, 4, P, C).transpose(0, 2, 1, 3)
            ).reshape(4, P, 4 * C)
            in_maps.append(
                {
                    "xT": xT_e,
                    "Wg": _prep_w_gu(Wg[e]),
                    "Wu": _prep_w_gu(Wu[e]),
                    "Wd": _prep_w_d(Wd[e]),
                }
            )
        res = run_bass_kernel_spmd(nc, in_maps, core_ids=list(range(E)))
        LAST_RESULTS = res
        for e in range(E):
            ix = parts[e]
            if len(ix) == 0:
                continue
            yT_e = res.results[e]["yT"]  # [H, C]
            out[ix] += route[ix, e][:, None] * yT_e[:, : len(ix)].T
    return out
